# revision 35
# baseline (speedup 1.0000x reference)
"""T5-style 4-layer encoder on 8 trn2 NeuronCores, tensor-parallel.

Sharding: heads (2/core) for attention, d_ff (352->384 padded /core) for FFN.
Two f16 AllReduces per layer (attn-out, ffn-out), each chunked in 2 so the
collective overlaps compute.

v2: feature-major residual stream. The residual h lives ONLY transposed
(hT, f16, [d, t]) so the per-layer PE transposes + PSUM evictions of the
old token-major scheme disappear. RMS stats come from sq(hT) reduced over
partitions by an all-ones matmul (result replicated across partitions),
rstd = exp(-0.5*ln(ssq/D+eps)) on ACT (ln+exp share one LUT set with the
attention exp, so no table reloads), and nT = hT * rstd is a 2x-mode DVE
multiply. AllReduce deltas return via transposed DMA (xbar) and are added
feature-major. The last layer only computes queries/FFN for the second
half of the sequence (the only half that reaches the output). Output
leaves feature-major; the host transposes.
"""
import os
import sys

sys.path.insert(0, "/opt/trn_rl_repo")

import numpy as np
import ml_dtypes

import concourse.bass as bass
import concourse.mybir as mybir
import concourse.tile as tile
from concourse.bass_utils import run_bass_kernel_spmd
from concourse.masks import make_identity

F32 = mybir.dt.float32
BF16 = mybir.dt.bfloat16
F16 = mybir.dt.float16
AF = mybir.ActivationFunctionType
OP = mybir.AluOpType

# model dims
L, D, H, DK, DFF, V = 4, 1024, 16, 64, 2816, 32128
B, S = 1, 1024
T = 2 * S
NB, MAXD = 32, 128
EPS = 1e-6

NCORES = 8
HL = H // NCORES          # 2 heads per core
DH = HL * DK              # 128 local head dims
FLR = DFF // NCORES       # 352 real local dff
FL = 384                  # padded local dff
F3 = FL // 128            # 3
TT = T // 128             # 16 token tiles
KD = D // 128             # 8 contraction tiles over D
QC = T // 512             # 4 query chunks
DC = D // 512             # 2 output-d chunks
RG = [list(range(NCORES))]

N_LAYERS = int(os.environ.get("KERN_LAYERS", str(L)))
CC_DTYPE = os.environ.get("KERN_CC_DTYPE", "f16")  # f16 | bf16 | f32
CC_DT = {"f16": F16, "bf16": BF16, "f32": F32}[CC_DTYPE]
CC_NP = {"f16": np.float16, "bf16": ml_dtypes.bfloat16, "f32": np.float32}[CC_DTYPE]
# last layer computes only second-half queries when running the full model
HALF_LAST = os.environ.get("KERN_HALF_LAST", "1") == "1" and N_LAYERS == L


def _fix_drain_waits(nc):
    """This walrus build has tight per-instruction sem-wait slot limits
    (Drain: none at all). Hoist excess waits onto preceding same-engine
    NoOps (<=2 each)."""
    for f in nc.m.functions:
        for bb in f.blocks:
            insts = bb.instructions
            i = 0
            while i < len(insts):
                ins = insts[i]
                si = ins.sync_info
                if si is None or not si.on_wait:
                    i += 1
                    continue
                keep = 0 if isinstance(ins, mybir.InstDrain) else 1
                waits = list(si.on_wait)
                if len(waits) <= keep:
                    i += 1
                    continue
                excess, kept = waits[: len(waits) - keep], waits[len(waits) - keep:]
                nops = [
                    mybir.InstNoOp(
                        name=f"{ins.name}_waitsplit{j}",
                        sync_info=mybir.SyncInfo(
                            on_wait=[w], on_update=[]
                        ),
                        bass_nofuse=True,
                        engine=ins.engine,
                    )
                    for j, w in enumerate(excess)
                ]
                ins.sync_info = mybir.SyncInfo(on_wait=kept, on_update=si.on_update)
                for k, nop in enumerate(nops):
                    insts.insert(i + k, nop)
                i += len(nops) + 1


def _layer_ranges(lyr):
    """(qc_lo, qc_hi, ti_lo, ti_hi) for this layer's query-side work."""
    if HALF_LAST and lyr == L - 1:
        return QC // 2, QC, TT // 2, TT
    return 0, QC, 0, TT


def build_program():
    nc = bass.Bass(num_devices=NCORES)

    h0T = nc.declare_dram_parameter("h0T", [D, T], F16, isOutput=False)
    WQ = nc.declare_dram_parameter("WQ", [L, D, DH], F16, isOutput=False)
    WK = nc.declare_dram_parameter("WK", [L, D, DH], F16, isOutput=False)
    WV = nc.declare_dram_parameter("WV", [L, D, DH], F16, isOutput=False)
    WO = nc.declare_dram_parameter("WO", [L, DH, D], BF16, isOutput=False)
    WI = nc.declare_dram_parameter("WI", [L, D, 2 * FL], F16, isOutput=False)
    WF = nc.declare_dram_parameter("WF", [L, FL, D], F16, isOutput=False)
    MT = nc.declare_dram_parameter("MT", [HL, 128, 3968], BF16, isOutput=False)
    out = nc.declare_dram_parameter("out", [D, S], F32, isOutput=True)

    # collective bounce buffers (plain dram tensors; pool tiles crash ENCD)
    ccin = {}
    ccout = {}
    ccrange = {}
    for name in ("attn", "ffn"):
        for lyr in range(N_LAYERS):
            _, _, ti_lo, ti_hi = _layer_ranges(lyr)
            half = (ti_hi - ti_lo) // 2
            for c in range(2):
                t0 = (ti_lo + c * half) * 128
                ntok = half * 128
                ccrange[(name, lyr, c)] = (t0, ntok)
                ccin[(name, lyr, c)] = nc.dram_tensor(
                    f"ccin_{name}_{lyr}_{c}", [ntok, D], CC_DT
                )
                ccout[(name, lyr, c)] = nc.dram_tensor(
                    f"ccout_{name}_{lyr}_{c}", [ntok, D], CC_DT, addr_space="Shared"
                )

    with tile.TileContext(nc) as tc:
        with (
            tc.tile_pool(name="pers", bufs=1) as pers,
            tc.tile_pool(name="wbuf", bufs=1) as wbuf,
            tc.tile_pool(name="stat", bufs=4) as stat,
            tc.tile_pool(name="sqp", bufs=3) as sqp,
            tc.tile_pool(name="ep", bufs=4) as ep,
            tc.tile_pool(name="arp", bufs=3) as arp,
            tc.tile_pool(name="resp", bufs=3) as resp,
            tc.tile_pool(name="ps_acc", bufs=3, space="PSUM") as ps_acc,
            tc.tile_pool(name="ps_tmp", bufs=3, space="PSUM") as ps_tmp,
            tc.tile_pool(name="ps_sm", bufs=2, space="PSUM") as ps_sm,
        ):
            # ---------- persistent SBUF ----------
            hT_sb = pers.tile([128, KD * 2048], F16)
            nT_sb = pers.tile([128, KD * 2048], F16)
            mt_sb = pers.tile([128, HL * 3968], BF16)
            ident = pers.tile([128, 128], F32)
            ones_sb = pers.tile([128, 128], F16)
            qT_sb = pers.tile([128, 2048], F16)
            kT_sb = pers.tile([128, 2048], F16)
            vaug_sb = pers.tile([128, TT * 130], BF16)
            oTu_sb = pers.tile([128, 2048], BF16)
            zrow_sb = [pers.tile([1, 2048], F32, name=f"zrow{i}") for i in range(HL)]
            rz_sb = pers.tile([128, TT * HL], F32)
            gT_sb = pers.tile([128, F3 * 2048], F16)

            wq_sb = wbuf.tile([128, KD * DH], F16)
            wk_sb = wbuf.tile([128, KD * DH], F16)
            wv_sb = wbuf.tile([128, KD * DH], F16)
            wo_sb = wbuf.tile([128, D], BF16)
            wi_sb = wbuf.tile([128, KD * 2 * FL], F16)
            wf_sb = wbuf.tile([128, F3 * D], F16)

            nc.vector.memset(ones_sb[:], 1.0)
            make_identity(nc, ident[:])
            eps_sb = pers.tile([128, 1], F32)
            nc.vector.memset(eps_sb[:], EPS)

            # load hT0 (feature-major), split per token-chunk so layer-0
            # rms1 starts after the first quarter lands
            for qc0 in range(QC):
                nc.sync.dma_start(
                    out=hT_sb[:].rearrange("p (kd t) -> p kd t", kd=KD)[
                        :, :, qc0 * 512:(qc0 + 1) * 512
                    ],
                    in_=h0T[:, qc0 * 512:(qc0 + 1) * 512].rearrange(
                        "(kd p) t -> p kd t", p=128
                    ),
                )
            nc.sync.dma_start(
                out=mt_sb[:].rearrange("p (hh c) -> p hh c", hh=HL),
                in_=MT[:].rearrange("hh p c -> p hh c"),
            )
            # ones columns of vaug (cols 64 and 129 of each 130-block)
            nc.vector.memset(
                vaug_sb[:].rearrange("p (ti j) -> p ti j", j=130)[:, :, 64:65], 1.0
            )
            nc.vector.memset(
                vaug_sb[:].rearrange("p (ti j) -> p ti j", j=130)[:, :, 129:130], 1.0
            )

            def rms_feature(dst, qc_lo, qc_hi):
                """hT (f16 feature-major) -> rms-normalized f16 nT chunks."""
                for qc in range(qc_lo, qc_hi):
                    ssq = ps_acc.tile([128, 512], F32, tag="pacc")
                    for kd in range(KD):
                        sl = slice(kd * 2048 + qc * 512, kd * 2048 + qc * 512 + 512)
                        sqt = sqp.tile([128, 512], F16, tag="sq")
                        nc.vector.tensor_mul(sqt[:], hT_sb[:, sl], hT_sb[:, sl])
                        nc.tensor.matmul(
                            ssq[:], lhsT=ones_sb[:], rhs=sqt[:],
                            start=(kd == 0), stop=(kd == KD - 1),
                        )
                    # rstd = 1/sqrt(ssq/D + eps) = exp(-0.5*ln(ssq/D + eps));
                    # ln+exp share one ACT table set with the attention exp,
                    # so no LUT reloads (sqrt lives in a different set)
                    lno = stat.tile([128, 512], F32, tag="lno")
                    nc.scalar.activation(
                        lno[:], ssq[:], AF.Ln, bias=eps_sb[:], scale=1.0 / D
                    )
                    rstd = stat.tile([128, 512], F16, tag="rstd")
                    nc.scalar.activation(rstd[:], lno[:], AF.Exp, scale=-0.5)
                    for kd in range(KD):
                        sl = slice(kd * 2048 + qc * 512, kd * 2048 + qc * 512 + 512)
                        nc.vector.tensor_mul(dst[:, sl], hT_sb[:, sl], rstd[:])

            def do_allreduce(name, lyr, c):
                """AllReduce chunk c + transposed residual add into hT."""
                if os.environ.get("KERN_NO_CC") == "1":
                    nc.sync.dma_start(
                        out=ccout[(name, lyr, c)][:], in_=ccin[(name, lyr, c)][:]
                    )
                else:
                    nc.gpsimd.collective_compute(
                        "AllReduce", OP.add, replica_groups=RG,
                        ins=[ccin[(name, lyr, c)][:].opt()],
                        outs=[ccout[(name, lyr, c)][:].opt()],
                    )
                t0, ntok = ccrange[(name, lyr, c)]
                for kd in range(KD):
                    dstg = resp.tile([128, 1024], CC_DT, tag="rstg")
                    nc.sync.dma_start_transpose(
                        dstg[:, 0:ntok], ccout[(name, lyr, c)][:, kd * 128:(kd + 1) * 128]
                    )
                    sl = slice(kd * 2048 + t0, kd * 2048 + t0 + ntok)
                    nc.vector.tensor_add(hT_sb[:, sl], hT_sb[:, sl], dstg[:, 0:ntok])

            for lyr in range(N_LAYERS):
                qc_lo, qc_hi, ti_lo, ti_hi = _layer_ranges(lyr)
                # ---------- load layer weights ----------
                nc.sync.dma_start(
                    out=wq_sb[:].rearrange("p (kd m) -> p kd m", kd=KD),
                    in_=WQ[lyr].rearrange("(kd p) m -> p kd m", p=128),
                )
                nc.sync.dma_start(
                    out=wk_sb[:].rearrange("p (kd m) -> p kd m", kd=KD),
                    in_=WK[lyr].rearrange("(kd p) m -> p kd m", p=128),
                )
                nc.sync.dma_start(
                    out=wv_sb[:].rearrange("p (kd m) -> p kd m", kd=KD),
                    in_=WV[lyr].rearrange("(kd p) m -> p kd m", p=128),
                )
                nc.sync.dma_start(out=wo_sb[:], in_=WO[lyr])
                nc.sync.dma_start(
                    out=wi_sb[:].rearrange("p (kd m) -> p kd m", kd=KD),
                    in_=WI[lyr].rearrange("(kd p) m -> p kd m", p=128),
                )
                nc.sync.dma_start(
                    out=wf_sb[:].rearrange("p (f3 m) -> p f3 m", f3=F3),
                    in_=WF[lyr].rearrange("(f3 p) m -> p f3 m", p=128),
                )

                # ---------- rms1 (full: K/V need all tokens) ----------
                rms_feature(nT_sb, 0, QC)

                # ---------- q/k projections (feature-major out) ----------
                for w_sb, dst, lo, hi in (
                    (wq_sb, qT_sb, qc_lo, qc_hi), (wk_sb, kT_sb, 0, QC)
                ):
                    for qc in range(lo, hi):
                        pq = ps_acc.tile([128, 512], F32, tag="pacc")
                        for kd in range(KD):
                            nc.tensor.matmul(
                                pq[:],
                                lhsT=w_sb[:, kd * DH:(kd + 1) * DH],
                                rhs=nT_sb[:, kd * 2048 + qc * 512: kd * 2048 + qc * 512 + 512],
                                start=(kd == 0), stop=(kd == KD - 1),
                            )
                        nc.scalar.copy(dst[:, qc * 512:(qc + 1) * 512], pq[:])

                # ---------- v projection (token-major into vaug) ----------
                for ti in range(TT):
                    pv = ps_sm.tile([128, 128], F32, tag="pt")
                    for kd in range(KD):
                        nc.tensor.matmul(
                            pv[:],
                            lhsT=nT_sb[:, kd * 2048 + ti * 128: kd * 2048 + (ti + 1) * 128],
                            rhs=wv_sb[:, kd * DH:(kd + 1) * DH],
                            start=(kd == 0), stop=(kd == KD - 1),
                        )
                    nc.scalar.copy(
                        vaug_sb[:, ti * 130: ti * 130 + 130]
                        .rearrange("p (j k) -> p j k", j=2, k=65)[:, :, 0:64],
                        pv[:].rearrange("p (j k) -> p j k", j=2, k=64),
                    )

                # ---------- attention (transposed scores; per local head) ----------
                for hh in range(HL):
                    mtb = hh * 3968
                    for qc in range(qc_lo, qc_hi):
                        po = ps_acc.tile([128, 512], F32, tag="pacc")
                        for kt in range(TT):
                            ps = ps_tmp.tile([128, 512], F32, tag="ptmp")
                            nc.tensor.matmul(
                                ps[:],
                                lhsT=kT_sb[hh * 64:(hh + 1) * 64, kt * 128:(kt + 1) * 128],
                                rhs=qT_sb[hh * 64:(hh + 1) * 64, qc * 512:(qc + 1) * 512],
                                start=True, stop=True,
                            )
                            e1 = ep.tile([128, 512], BF16, tag="e1")
                            nc.scalar.activation(e1[:], ps[:], AF.Exp)
                            off = mtb + qc * 512 - kt * 128 + 1920
                            nc.vector.tensor_mul(e1[:], e1[:], mt_sb[:, off: off + 512])
                            nc.tensor.matmul(
                                po[0:65, :],
                                lhsT=vaug_sb[:, kt * 130 + hh * 65: kt * 130 + hh * 65 + 65],
                                rhs=e1[:],
                                start=(kt == 0), stop=(kt == TT - 1),
                                skip_group_check=True,
                            )
                        nc.scalar.copy(
                            oTu_sb[hh * 64:(hh + 1) * 64, qc * 512:(qc + 1) * 512],
                            po[0:64, :],
                        )
                        nc.scalar.copy(
                            zrow_sb[hh][:, qc * 512:(qc + 1) * 512], po[64:65, :]
                        )

                # ---------- Z transpose + reciprocal ----------
                for ti in range(ti_lo, ti_hi):
                    pz = ps_sm.tile([128, 128], F32, tag="pt")
                    for hh in range(HL):
                        nc.tensor.matmul(
                            pz[0:128, hh:hh + 1],
                            lhsT=zrow_sb[hh][0:1, ti * 128:(ti + 1) * 128],
                            rhs=ident[0:1, 0:1],
                            start=True, stop=True,
                            skip_group_check=True,
                        )
                    nc.vector.reciprocal(
                        rz_sb[:, ti * HL:(ti + 1) * HL], pz[0:128, 0:HL]
                    )

                # ---------- Wo (+1/Z fold) -> attn partial -> AR ----------
                half = (ti_hi - ti_lo) // 2
                for c in range(2):
                    tis = range(ti_lo + c * half, ti_lo + (c + 1) * half)
                    for ti in tis:
                        rzA = rz_sb[:, ti * HL: ti * HL + 1]
                        rzB = rz_sb[:, ti * HL + 1: ti * HL + 2]
                        for dc in range(DC):
                            pA = ps_tmp.tile([128, 512], F32, tag="ptmp")
                            pB = ps_tmp.tile([128, 512], F32, tag="ptmp")
                            nc.tensor.matmul(
                                pA[:],
                                lhsT=oTu_sb[0:64, ti * 128:(ti + 1) * 128],
                                rhs=wo_sb[0:64, dc * 512:(dc + 1) * 512],
                                start=True, stop=True,
                            )
                            nc.tensor.matmul(
                                pB[:],
                                lhsT=oTu_sb[64:128, ti * 128:(ti + 1) * 128],
                                rhs=wo_sb[64:128, dc * 512:(dc + 1) * 512],
                                start=True, stop=True,
                            )
                            u = arp.tile([128, 512], F32, tag="u", bufs=2)
                            nc.vector.tensor_scalar_mul(u[:], pB[:], rzB)
                            a = arp.tile([128, 512], CC_DT, tag="a")
                            nc.vector.scalar_tensor_tensor(
                                a[:], in0=pA[:], scalar=rzA, in1=u[:],
                                op0=OP.mult, op1=OP.add,
                            )
                            nc.sync.dma_start(
                                out=ccin[("attn", lyr, c)][
                                    (ti - ti_lo - c * half) * 128:
                                    (ti - ti_lo - c * half + 1) * 128,
                                    dc * 512:(dc + 1) * 512,
                                ],
                                in_=a[:],
                            )
                    do_allreduce("attn", lyr, c)

                # ---------- rms2 (query range only) ----------
                rms_feature(nT_sb, qc_lo, qc_hi)

                # ---------- FFN up (wi0|wi1), gelu*gate -> gT (feature-major) ----------
                for f3 in range(F3):
                    for qc in range(qc_lo, qc_hi):
                        pg0 = ps_acc.tile([128, 512], F32, tag="pacc")
                        pg1 = ps_tmp.tile([128, 512], F32, tag="ptmp")
                        for kd in range(KD):
                            rhs = nT_sb[:, kd * 2048 + qc * 512: kd * 2048 + qc * 512 + 512]
                            nc.tensor.matmul(
                                pg0[:],
                                lhsT=wi_sb[:, kd * 2 * FL + f3 * 128: kd * 2 * FL + (f3 + 1) * 128],
                                rhs=rhs,
                                start=(kd == 0), stop=(kd == KD - 1),
                                skip_group_check=True,
                            )
                            nc.tensor.matmul(
                                pg1[:],
                                lhsT=wi_sb[:, kd * 2 * FL + FL + f3 * 128: kd * 2 * FL + FL + (f3 + 1) * 128],
                                rhs=rhs,
                                start=(kd == 0), stop=(kd == KD - 1),
                                skip_group_check=True,
                            )
                        gt = ep.tile([128, 512], F32, tag="gt", bufs=2)
                        nc.scalar.activation(gt[:], pg0[:], AF.Gelu_apprx_tanh)
                        nc.vector.tensor_mul(
                            gT_sb[:, f3 * 2048 + qc * 512: f3 * 2048 + (qc + 1) * 512],
                            gt[:], pg1[:],
                        )

                # ---------- FFN down -> partial -> AR ----------
                for c in range(2):
                    tis = range(ti_lo + c * half, ti_lo + (c + 1) * half)
                    for i, ti in enumerate(tis):
                        for dc in range(DC):
                            pf = ps_tmp.tile([128, 512], F32, tag="ptmp")
                            for f3 in range(F3):
                                nc.tensor.matmul(
                                    pf[:],
                                    lhsT=gT_sb[:, f3 * 2048 + ti * 128: f3 * 2048 + (ti + 1) * 128],
                                    rhs=wf_sb[:, f3 * 1024 + dc * 512: f3 * 1024 + (dc + 1) * 512],
                                    start=(f3 == 0), stop=(f3 == F3 - 1),
                                )
                            a = arp.tile([128, 512], CC_DT, tag="a")
                            nc.scalar.copy(a[:], pf[:])
                            nc.sync.dma_start(
                                out=ccin[("ffn", lyr, c)][
                                    i * 128:(i + 1) * 128,
                                    dc * 512:(dc + 1) * 512,
                                ],
                                in_=a[:],
                            )
                    do_allreduce("ffn", lyr, c)

            # ---------- final rms on second half, output (feature-major) ----------
            for qc in range(QC // 2, QC):
                ssq = ps_acc.tile([128, 512], F32, tag="pacc")
                for kd in range(KD):
                    sl = slice(kd * 2048 + qc * 512, kd * 2048 + qc * 512 + 512)
                    sqt = sqp.tile([128, 512], F16, tag="sq")
                    nc.vector.tensor_mul(sqt[:], hT_sb[:, sl], hT_sb[:, sl])
                    nc.tensor.matmul(
                        ssq[:], lhsT=ones_sb[:], rhs=sqt[:],
                        start=(kd == 0), stop=(kd == KD - 1),
                    )
                lno = stat.tile([128, 512], F32, tag="lno")
                nc.scalar.activation(
                    lno[:], ssq[:], AF.Ln, bias=eps_sb[:], scale=1.0 / D
                )
                rstd = stat.tile([128, 512], F16, tag="rstd")
                nc.scalar.activation(rstd[:], lno[:], AF.Exp, scale=-0.5)
                for kd in range(KD):
                    sl = slice(kd * 2048 + qc * 512, kd * 2048 + qc * 512 + 512)
                    o = arp.tile([128, 512], F32, tag="u", bufs=2)
                    nc.vector.tensor_mul(o[:], hT_sb[:, sl], rstd[:])
                    nc.sync.dma_start(
                        out=out[kd * 128:(kd + 1) * 128,
                                (qc - QC // 2) * 512:(qc - QC // 2 + 1) * 512],
                        in_=o[:],
                    )

    _fix_drain_waits(nc)
    return nc


# ---------------- host side ----------------

def _rel_bucket_np(rel):
    """numpy replica of reference _rel_bucket (int32/float32 semantics)."""
    nb = NB // 2
    ret = (rel > 0).astype(np.int32) * nb
    arel = np.abs(rel)
    max_exact = nb // 2
    t = np.log(np.maximum(arel, 1).astype(np.float32) / np.float32(max_exact))
    t = t / np.float32(np.log(MAXD / max_exact)) * np.float32(nb - max_exact)
    large = max_exact + t.astype(np.int32)
    large = np.minimum(large, nb - 1)
    return ret + np.where(arel < max_exact, arel.astype(np.int32), large)


def _build_mt(rel_bias, core):
    """exp(bias) master table [HL, 128, 3968] for this core's heads."""
    d = np.arange(-(T - 1), T, dtype=np.int64)          # k - q in [-2047, 2047]
    buckets = _rel_bucket_np(d)                          # [4095]
    p = np.arange(128)[:, None]
    i = np.arange(3968)[None, :]
    idx = 3967 + p - i                                   # in [0, 4094]
    mts = []
    for hh in range(HL):
        head = core * HL + hh
        toep = rel_bias[buckets, head].astype(np.float32)  # [4095]
        mts.append(np.exp(toep)[idx])
    return np.stack(mts).astype(ml_dtypes.bfloat16)


_prog_cache = {}


def kernel(**inputs):
    input_ids = np.asarray(inputs["input_ids"]).astype(np.int64)
    memory = np.asarray(inputs["memory"], dtype=np.float32)
    embed = np.asarray(inputs["embed"], dtype=np.float32)
    Wq = np.asarray(inputs["Wq"], dtype=np.float32)
    Wk = np.asarray(inputs["Wk"], dtype=np.float32)
    Wv = np.asarray(inputs["Wv"], dtype=np.float32)
    Wo = np.asarray(inputs["Wo"], dtype=np.float32)
    ln1 = np.asarray(inputs["ln1"], dtype=np.float32)
    ln2 = np.asarray(inputs["ln2"], dtype=np.float32)
    wi0 = np.asarray(inputs["wi0"], dtype=np.float32)
    wi1 = np.asarray(inputs["wi1"], dtype=np.float32)
    wo_ff = np.asarray(inputs["wo_ff"], dtype=np.float32)
    final_ln = np.asarray(inputs["final_ln"], dtype=np.float32)
    rel_bias = np.asarray(inputs["rel_bias"], dtype=np.float32)

    bf = np.float16

    x = embed[input_ids[0]]                      # [S, D]
    h0 = np.concatenate([memory[0], x], axis=0)  # [T, D] f32
    h0T = np.ascontiguousarray(h0.T).astype(bf)  # [D, T] f16

    in_maps = []
    for c in range(NCORES):
        hs = slice(c * DH, (c + 1) * DH)
        fs = slice(c * FLR, (c + 1) * FLR)
        wq_c = (ln1[:, :, None] * Wq)[:, :, hs].astype(bf)          # [L, D, DH]
        wk_c = (ln1[:, :, None] * Wk)[:, :, hs].astype(bf)
        wv_c = (ln1[:, :, None] * Wv)[:, :, hs].astype(bf)
        wo_c = Wo[:, hs, :].astype(ml_dtypes.bfloat16)               # [L, DH, D]
        wi_c = np.zeros((L, D, 2 * FL), np.float32)
        wi_c[:, :, :FLR] = (ln2[:, :, None] * wi0)[:, :, fs]
        wi_c[:, :, FL:FL + FLR] = (ln2[:, :, None] * wi1)[:, :, fs]
        wf_c = np.zeros((L, FL, D), np.float32)
        wf_c[:, :FLR, :] = wo_ff[:, fs, :]
        in_maps.append({
            "h0T": h0T,
            "WQ": np.ascontiguousarray(wq_c),
            "WK": np.ascontiguousarray(wk_c),
            "WV": np.ascontiguousarray(wv_c),
            "WO": np.ascontiguousarray(wo_c),
            "WI": wi_c.astype(bf),
            "WF": wf_c.astype(bf),
            "MT": _build_mt(rel_bias, c),
        })

    if "nc" not in _prog_cache:
        _prog_cache["nc"] = build_program()
    nc = _prog_cache["nc"]
    _prog_cache["in_maps"] = in_maps

    res = run_bass_kernel_spmd(nc, in_maps, list(range(NCORES)))
    hidT = res.results[0]["out"]                 # [D, S] normalized, unweighted
    outp = hidT.T * final_ln[None, :] + memory[0]
    return outp[None].astype(np.float32)


if __name__ == "__main__":
    rng = np.random.default_rng(0)
    fake = {
        "input_ids": rng.integers(0, V, (B, S)),
        "memory": rng.standard_normal((B, S, D), dtype=np.float32),
        "embed": rng.standard_normal((V, D), dtype=np.float32) * 0.02,
        "Wq": rng.standard_normal((L, D, H * DK), dtype=np.float32) * 0.02,
        "Wk": rng.standard_normal((L, D, H * DK), dtype=np.float32) * 0.02,
        "Wv": rng.standard_normal((L, D, H * DK), dtype=np.float32) * 0.02,
        "Wo": rng.standard_normal((L, H * DK, D), dtype=np.float32) * 0.02,
        "ln1": np.ones((L, D), np.float32),
        "ln2": np.ones((L, D), np.float32),
        "wi0": rng.standard_normal((L, D, DFF), dtype=np.float32) * 0.02,
        "wi1": rng.standard_normal((L, D, DFF), dtype=np.float32) * 0.02,
        "wo_ff": rng.standard_normal((L, DFF, D), dtype=np.float32) * 0.02,
        "final_ln": np.ones((D,), np.float32),
        "rel_bias": rng.standard_normal((NB, H), dtype=np.float32) * 0.02,
    }
    o = kernel(**fake)
    print("out", o.shape, o.dtype, np.abs(o).mean())


# revision 36
# speedup vs baseline: 1.2025x; 1.2025x over previous
"""T5-style 4-layer encoder on 8 trn2 NeuronCores, tensor-parallel.

Sharding: heads (2/core) for attention, d_ff (352->384 padded /core) for FFN.
Two f16 AllReduces per layer (attn-out, ffn-out), each chunked in 2 so the
collective overlaps compute.

v2: feature-major residual stream. The residual h lives ONLY transposed
(hT, f16, [d, t]) so the per-layer PE transposes + PSUM evictions of the
old token-major scheme disappear. RMS stats come from sq(hT) reduced over
partitions by an all-ones matmul (result replicated across partitions),
rstd = exp(-0.5*ln(ssq/D+eps)) on ACT (ln+exp share one LUT set with the
attention exp, so no table reloads), and nT = hT * rstd is a 2x-mode DVE
multiply. AllReduce deltas return via transposed DMA (xbar) and are added
feature-major. The last layer only computes queries/FFN for the second
half of the sequence (the only half that reaches the output). Output
leaves feature-major; the host transposes.
"""
import os
import sys

sys.path.insert(0, "/opt/trn_rl_repo")

import numpy as np
import ml_dtypes

import concourse.bass as bass
import concourse.mybir as mybir
import concourse.tile as tile
from concourse.bass_utils import run_bass_kernel_spmd
from concourse.masks import make_identity

F32 = mybir.dt.float32
BF16 = mybir.dt.bfloat16
F16 = mybir.dt.float16
AF = mybir.ActivationFunctionType
OP = mybir.AluOpType

# model dims
L, D, H, DK, DFF, V = 4, 1024, 16, 64, 2816, 32128
B, S = 1, 1024
T = 2 * S
NB, MAXD = 32, 128
EPS = 1e-6

NCORES = 8
HL = H // NCORES          # 2 heads per core
DH = HL * DK              # 128 local head dims
FLR = DFF // NCORES       # 352 real local dff
FL = 384                  # padded local dff
F3 = FL // 128            # 3
TT = T // 128             # 16 token tiles
KD = D // 128             # 8 contraction tiles over D
QC = T // 512             # 4 query chunks
DC = D // 512             # 2 output-d chunks
RG = [list(range(NCORES))]

N_LAYERS = int(os.environ.get("KERN_LAYERS", str(L)))
CC_DTYPE = os.environ.get("KERN_CC_DTYPE", "f16")  # f16 | bf16 | f32
CC_DT = {"f16": F16, "bf16": BF16, "f32": F32}[CC_DTYPE]
CC_NP = {"f16": np.float16, "bf16": ml_dtypes.bfloat16, "f32": np.float32}[CC_DTYPE]
# last layer computes only second-half queries when running the full model
HALF_LAST = os.environ.get("KERN_HALF_LAST", "1") == "1" and N_LAYERS == L


def _fix_drain_waits(nc):
    """This walrus build has tight per-instruction sem-wait slot limits
    (Drain: none at all). Hoist excess waits onto preceding same-engine
    NoOps (<=2 each)."""
    for f in nc.m.functions:
        for bb in f.blocks:
            insts = bb.instructions
            i = 0
            while i < len(insts):
                ins = insts[i]
                si = ins.sync_info
                if si is None or not si.on_wait:
                    i += 1
                    continue
                keep = 0 if isinstance(ins, mybir.InstDrain) else 1
                waits = list(si.on_wait)
                if len(waits) <= keep:
                    i += 1
                    continue
                excess, kept = waits[: len(waits) - keep], waits[len(waits) - keep:]
                nops = [
                    mybir.InstNoOp(
                        name=f"{ins.name}_waitsplit{j}",
                        sync_info=mybir.SyncInfo(
                            on_wait=[w], on_update=[]
                        ),
                        bass_nofuse=True,
                        engine=ins.engine,
                    )
                    for j, w in enumerate(excess)
                ]
                ins.sync_info = mybir.SyncInfo(on_wait=kept, on_update=si.on_update)
                for k, nop in enumerate(nops):
                    insts.insert(i + k, nop)
                i += len(nops) + 1


def _layer_ranges(lyr):
    """(qc_lo, qc_hi, ti_lo, ti_hi) for this layer's query-side work."""
    if HALF_LAST and lyr == L - 1:
        return QC // 2, QC, TT // 2, TT
    return 0, QC, 0, TT


def build_program():
    nc = bass.Bass(num_devices=NCORES)

    h0T = nc.declare_dram_parameter("h0T", [D, T], F16, isOutput=False)
    WQ = nc.declare_dram_parameter("WQ", [L, D, DH], F16, isOutput=False)
    WK = nc.declare_dram_parameter("WK", [L, D, DH], F16, isOutput=False)
    WV = nc.declare_dram_parameter("WV", [L, D, DH], F16, isOutput=False)
    WO = nc.declare_dram_parameter("WO", [L, DH, D], BF16, isOutput=False)
    WI = nc.declare_dram_parameter("WI", [L, D, 2 * FL], F16, isOutput=False)
    WF = nc.declare_dram_parameter("WF", [L, FL, D], F16, isOutput=False)
    MT = nc.declare_dram_parameter("MT", [HL, 128, 3968], BF16, isOutput=False)
    out = nc.declare_dram_parameter("out", [D, S], F32, isOutput=True)

    # collective bounce buffers (plain dram tensors; pool tiles crash ENCD)
    ccin = {}
    ccout = {}
    ccrange = {}
    for name in ("attn", "ffn"):
        for lyr in range(N_LAYERS):
            _, _, ti_lo, ti_hi = _layer_ranges(lyr)
            half = (ti_hi - ti_lo) // 2
            for c in range(2):
                t0 = (ti_lo + c * half) * 128
                ntok = half * 128
                ccrange[(name, lyr, c)] = (t0, ntok)
                ccin[(name, lyr, c)] = nc.dram_tensor(
                    f"ccin_{name}_{lyr}_{c}", [ntok, D], CC_DT
                )
                ccout[(name, lyr, c)] = nc.dram_tensor(
                    f"ccout_{name}_{lyr}_{c}", [ntok, D], CC_DT, addr_space="Shared"
                )

    with tile.TileContext(nc) as tc:
        with (
            tc.tile_pool(name="pers", bufs=1) as pers,
            tc.tile_pool(name="wbuf", bufs=1) as wbuf,
            tc.tile_pool(name="stat", bufs=4) as stat,
            tc.tile_pool(name="sqp", bufs=4) as sqp,
            tc.tile_pool(name="ep", bufs=6) as ep,
            tc.tile_pool(name="arp", bufs=3) as arp,
            tc.tile_pool(name="resp", bufs=3) as resp,
            tc.tile_pool(name="ps_acc", bufs=3, space="PSUM") as ps_acc,
            tc.tile_pool(name="ps_tmp", bufs=4, space="PSUM") as ps_tmp,
            tc.tile_pool(name="ps_sm", bufs=1, space="PSUM") as ps_sm,
        ):
            # ---------- persistent SBUF ----------
            hT_sb = pers.tile([128, KD * 2048], F16)
            nT_sb = pers.tile([128, KD * 2048], F16)
            mt_sb = pers.tile([128, HL * 3968], BF16)
            ident = pers.tile([128, 128], F32)
            ones_sb = pers.tile([128, 128], F16)
            qT_sb = pers.tile([128, 2048], F16)
            kT_sb = pers.tile([128, 2048], F16)
            vaug_sb = pers.tile([128, TT * 130], BF16)
            oTu_sb = pers.tile([128, 2048], BF16)
            zrow_sb = [pers.tile([1, 2048], F32, name=f"zrow{i}") for i in range(HL)]
            rz_sb = pers.tile([128, TT * HL], F32)
            gT_sb = pers.tile([128, F3 * 2048], F16)

            wq_sb = wbuf.tile([128, KD * DH], F16)
            wk_sb = wbuf.tile([128, KD * DH], F16)
            wv_sb = wbuf.tile([128, KD * DH], F16)
            wo_sb = wbuf.tile([128, D], BF16)
            wi_sb = wbuf.tile([128, KD * 2 * FL], F16)
            wf_sb = wbuf.tile([128, F3 * D], F16)

            nc.vector.memset(ones_sb[:], 1.0)
            make_identity(nc, ident[:])
            eps_sb = pers.tile([128, 1], F32)
            nc.vector.memset(eps_sb[:], EPS)

            # load hT0 (feature-major), split per token-chunk so layer-0
            # rms1 starts after the first quarter lands
            for qc0 in range(QC):
                nc.sync.dma_start(
                    out=hT_sb[:].rearrange("p (kd t) -> p kd t", kd=KD)[
                        :, :, qc0 * 512:(qc0 + 1) * 512
                    ],
                    in_=h0T[:, qc0 * 512:(qc0 + 1) * 512].rearrange(
                        "(kd p) t -> p kd t", p=128
                    ),
                )
            nc.sync.dma_start(
                out=mt_sb[:].rearrange("p (hh c) -> p hh c", hh=HL),
                in_=MT[:].rearrange("hh p c -> p hh c"),
            )
            # ones columns of vaug (cols 64 and 129 of each 130-block)
            nc.vector.memset(
                vaug_sb[:].rearrange("p (ti j) -> p ti j", j=130)[:, :, 64:65], 1.0
            )
            nc.vector.memset(
                vaug_sb[:].rearrange("p (ti j) -> p ti j", j=130)[:, :, 129:130], 1.0
            )

            def rms_feature(dst, qc_lo, qc_hi):
                """hT (f16 feature-major) -> rms-normalized f16 nT chunks."""
                for qc in range(qc_lo, qc_hi):
                    ssq = ps_acc.tile([128, 512], F32, tag="pacc")
                    for kd in range(KD):
                        sl = slice(kd * 2048 + qc * 512, kd * 2048 + qc * 512 + 512)
                        sqt = sqp.tile([128, 512], F16, tag="sq")
                        nc.vector.tensor_mul(sqt[:], hT_sb[:, sl], hT_sb[:, sl])
                        nc.tensor.matmul(
                            ssq[:], lhsT=ones_sb[:], rhs=sqt[:],
                            start=(kd == 0), stop=(kd == KD - 1),
                        )
                    # rstd = 1/sqrt(ssq/D + eps) = exp(-0.5*ln(ssq/D + eps));
                    # ln+exp share one ACT table set with the attention exp,
                    # so no LUT reloads (sqrt lives in a different set)
                    lno = stat.tile([128, 512], F32, tag="lno")
                    nc.scalar.activation(
                        lno[:], ssq[:], AF.Ln, bias=eps_sb[:], scale=1.0 / D
                    )
                    rstd = stat.tile([128, 512], F16, tag="rstd")
                    nc.scalar.activation(rstd[:], lno[:], AF.Exp, scale=-0.5)
                    for kd in range(KD):
                        sl = slice(kd * 2048 + qc * 512, kd * 2048 + qc * 512 + 512)
                        nc.vector.tensor_mul(dst[:, sl], hT_sb[:, sl], rstd[:])

            def do_allreduce(name, lyr, c):
                """AllReduce chunk c + transposed residual add into hT."""
                if os.environ.get("KERN_NO_CC") == "1":
                    nc.sync.dma_start(
                        out=ccout[(name, lyr, c)][:], in_=ccin[(name, lyr, c)][:]
                    )
                else:
                    nc.gpsimd.collective_compute(
                        "AllReduce", OP.add, replica_groups=RG,
                        ins=[ccin[(name, lyr, c)][:].opt()],
                        outs=[ccout[(name, lyr, c)][:].opt()],
                    )
                t0, ntok = ccrange[(name, lyr, c)]
                for kd in range(KD):
                    dstg = resp.tile([128, 1024], CC_DT, tag="rstg")
                    nc.sync.dma_start_transpose(
                        dstg[:, 0:ntok], ccout[(name, lyr, c)][:, kd * 128:(kd + 1) * 128]
                    )
                    sl = slice(kd * 2048 + t0, kd * 2048 + t0 + ntok)
                    nc.vector.tensor_add(hT_sb[:, sl], hT_sb[:, sl], dstg[:, 0:ntok])

            for lyr in range(N_LAYERS):
                qc_lo, qc_hi, ti_lo, ti_hi = _layer_ranges(lyr)
                # ---------- load layer weights ----------
                nc.sync.dma_start(
                    out=wq_sb[:].rearrange("p (kd m) -> p kd m", kd=KD),
                    in_=WQ[lyr].rearrange("(kd p) m -> p kd m", p=128),
                )
                nc.sync.dma_start(
                    out=wk_sb[:].rearrange("p (kd m) -> p kd m", kd=KD),
                    in_=WK[lyr].rearrange("(kd p) m -> p kd m", p=128),
                )
                nc.sync.dma_start(
                    out=wv_sb[:].rearrange("p (kd m) -> p kd m", kd=KD),
                    in_=WV[lyr].rearrange("(kd p) m -> p kd m", p=128),
                )
                nc.sync.dma_start(out=wo_sb[:], in_=WO[lyr])
                nc.sync.dma_start(
                    out=wi_sb[:].rearrange("p (kd m) -> p kd m", kd=KD),
                    in_=WI[lyr].rearrange("(kd p) m -> p kd m", p=128),
                )
                nc.sync.dma_start(
                    out=wf_sb[:].rearrange("p (f3 m) -> p f3 m", f3=F3),
                    in_=WF[lyr].rearrange("(f3 p) m -> p f3 m", p=128),
                )

                # ---------- rms1 (full: K/V need all tokens) ----------
                rms_feature(nT_sb, 0, QC)

                # ---------- q/k projections (feature-major out) ----------
                for w_sb, dst, lo, hi in (
                    (wq_sb, qT_sb, qc_lo, qc_hi), (wk_sb, kT_sb, 0, QC)
                ):
                    for qc in range(lo, hi):
                        pq = ps_acc.tile([128, 512], F32, tag="pacc")
                        for kd in range(KD):
                            nc.tensor.matmul(
                                pq[:],
                                lhsT=w_sb[:, kd * DH:(kd + 1) * DH],
                                rhs=nT_sb[:, kd * 2048 + qc * 512: kd * 2048 + qc * 512 + 512],
                                start=(kd == 0), stop=(kd == KD - 1),
                            )
                        nc.scalar.copy(dst[:, qc * 512:(qc + 1) * 512], pq[:])

                # ---------- v projection (token-major into vaug) ----------
                for ti in range(TT):
                    pv = ps_sm.tile([128, 128], F32, tag="pt")
                    for kd in range(KD):
                        nc.tensor.matmul(
                            pv[:],
                            lhsT=nT_sb[:, kd * 2048 + ti * 128: kd * 2048 + (ti + 1) * 128],
                            rhs=wv_sb[:, kd * DH:(kd + 1) * DH],
                            start=(kd == 0), stop=(kd == KD - 1),
                        )
                    nc.scalar.copy(
                        vaug_sb[:, ti * 130: ti * 130 + 130]
                        .rearrange("p (j k) -> p j k", j=2, k=65)[:, :, 0:64],
                        pv[:].rearrange("p (j k) -> p j k", j=2, k=64),
                    )

                # ---------- attention (transposed scores; per local head) ----------
                for hh in range(HL):
                    mtb = hh * 3968
                    for qc in range(qc_lo, qc_hi):
                        po = ps_acc.tile([128, 512], F32, tag="pacc")
                        for kt in range(TT):
                            ps = ps_tmp.tile([128, 512], F32, tag="ptmp")
                            nc.tensor.matmul(
                                ps[:],
                                lhsT=kT_sb[hh * 64:(hh + 1) * 64, kt * 128:(kt + 1) * 128],
                                rhs=qT_sb[hh * 64:(hh + 1) * 64, qc * 512:(qc + 1) * 512],
                                start=True, stop=True,
                            )
                            e1 = ep.tile([128, 512], BF16, tag="e1")
                            nc.scalar.activation(e1[:], ps[:], AF.Exp)
                            off = mtb + qc * 512 - kt * 128 + 1920
                            nc.vector.tensor_mul(e1[:], e1[:], mt_sb[:, off: off + 512])
                            nc.tensor.matmul(
                                po[0:65, :],
                                lhsT=vaug_sb[:, kt * 130 + hh * 65: kt * 130 + hh * 65 + 65],
                                rhs=e1[:],
                                start=(kt == 0), stop=(kt == TT - 1),
                                skip_group_check=True,
                            )
                        nc.scalar.copy(
                            oTu_sb[hh * 64:(hh + 1) * 64, qc * 512:(qc + 1) * 512],
                            po[0:64, :],
                        )
                        nc.scalar.copy(
                            zrow_sb[hh][:, qc * 512:(qc + 1) * 512], po[64:65, :]
                        )

                # ---------- Z transpose + reciprocal ----------
                for ti in range(ti_lo, ti_hi):
                    pz = ps_sm.tile([128, 128], F32, tag="pt")
                    for hh in range(HL):
                        nc.tensor.matmul(
                            pz[0:128, hh:hh + 1],
                            lhsT=zrow_sb[hh][0:1, ti * 128:(ti + 1) * 128],
                            rhs=ident[0:1, 0:1],
                            start=True, stop=True,
                            skip_group_check=True,
                        )
                    nc.vector.reciprocal(
                        rz_sb[:, ti * HL:(ti + 1) * HL], pz[0:128, 0:HL]
                    )

                # ---------- Wo (+1/Z fold) -> attn partial -> AR ----------
                half = (ti_hi - ti_lo) // 2
                for c in range(2):
                    tis = range(ti_lo + c * half, ti_lo + (c + 1) * half)
                    for ti in tis:
                        rzA = rz_sb[:, ti * HL: ti * HL + 1]
                        rzB = rz_sb[:, ti * HL + 1: ti * HL + 2]
                        for dc in range(DC):
                            pA = ps_tmp.tile([128, 512], F32, tag="ptmp")
                            pB = ps_tmp.tile([128, 512], F32, tag="ptmp")
                            nc.tensor.matmul(
                                pA[:],
                                lhsT=oTu_sb[0:64, ti * 128:(ti + 1) * 128],
                                rhs=wo_sb[0:64, dc * 512:(dc + 1) * 512],
                                start=True, stop=True,
                            )
                            nc.tensor.matmul(
                                pB[:],
                                lhsT=oTu_sb[64:128, ti * 128:(ti + 1) * 128],
                                rhs=wo_sb[64:128, dc * 512:(dc + 1) * 512],
                                start=True, stop=True,
                            )
                            u = arp.tile([128, 512], F32, tag="u", bufs=2)
                            nc.vector.tensor_scalar_mul(u[:], pB[:], rzB)
                            a = arp.tile([128, 512], CC_DT, tag="a")
                            nc.vector.scalar_tensor_tensor(
                                a[:], in0=pA[:], scalar=rzA, in1=u[:],
                                op0=OP.mult, op1=OP.add,
                            )
                            nc.sync.dma_start(
                                out=ccin[("attn", lyr, c)][
                                    (ti - ti_lo - c * half) * 128:
                                    (ti - ti_lo - c * half + 1) * 128,
                                    dc * 512:(dc + 1) * 512,
                                ],
                                in_=a[:],
                            )
                    do_allreduce("attn", lyr, c)

                # ---------- rms2 (query range only) ----------
                rms_feature(nT_sb, qc_lo, qc_hi)

                # ---------- FFN up (wi0|wi1), gelu*gate -> gT (feature-major) ----------
                for f3 in range(F3):
                    for qc in range(qc_lo, qc_hi):
                        pg0 = ps_acc.tile([128, 512], F32, tag="pacc")
                        pg1 = ps_tmp.tile([128, 512], F32, tag="ptmp")
                        for kd in range(KD):
                            rhs = nT_sb[:, kd * 2048 + qc * 512: kd * 2048 + qc * 512 + 512]
                            nc.tensor.matmul(
                                pg0[:],
                                lhsT=wi_sb[:, kd * 2 * FL + f3 * 128: kd * 2 * FL + (f3 + 1) * 128],
                                rhs=rhs,
                                start=(kd == 0), stop=(kd == KD - 1),
                                skip_group_check=True,
                            )
                            nc.tensor.matmul(
                                pg1[:],
                                lhsT=wi_sb[:, kd * 2 * FL + FL + f3 * 128: kd * 2 * FL + FL + (f3 + 1) * 128],
                                rhs=rhs,
                                start=(kd == 0), stop=(kd == KD - 1),
                                skip_group_check=True,
                            )
                        gt = ep.tile([128, 512], F32, tag="gt", bufs=2)
                        nc.scalar.activation(gt[:], pg0[:], AF.Gelu_apprx_tanh)
                        nc.vector.tensor_mul(
                            gT_sb[:, f3 * 2048 + qc * 512: f3 * 2048 + (qc + 1) * 512],
                            gt[:], pg1[:],
                        )

                # ---------- FFN down -> partial -> AR ----------
                for c in range(2):
                    tis = range(ti_lo + c * half, ti_lo + (c + 1) * half)
                    for i, ti in enumerate(tis):
                        for dc in range(DC):
                            pf = ps_tmp.tile([128, 512], F32, tag="ptmp")
                            for f3 in range(F3):
                                nc.tensor.matmul(
                                    pf[:],
                                    lhsT=gT_sb[:, f3 * 2048 + ti * 128: f3 * 2048 + (ti + 1) * 128],
                                    rhs=wf_sb[:, f3 * 1024 + dc * 512: f3 * 1024 + (dc + 1) * 512],
                                    start=(f3 == 0), stop=(f3 == F3 - 1),
                                )
                            a = arp.tile([128, 512], CC_DT, tag="a")
                            nc.scalar.copy(a[:], pf[:])
                            nc.sync.dma_start(
                                out=ccin[("ffn", lyr, c)][
                                    i * 128:(i + 1) * 128,
                                    dc * 512:(dc + 1) * 512,
                                ],
                                in_=a[:],
                            )
                    do_allreduce("ffn", lyr, c)

            # ---------- final rms on second half, output (feature-major) ----------
            for qc in range(QC // 2, QC):
                ssq = ps_acc.tile([128, 512], F32, tag="pacc")
                for kd in range(KD):
                    sl = slice(kd * 2048 + qc * 512, kd * 2048 + qc * 512 + 512)
                    sqt = sqp.tile([128, 512], F16, tag="sq")
                    nc.vector.tensor_mul(sqt[:], hT_sb[:, sl], hT_sb[:, sl])
                    nc.tensor.matmul(
                        ssq[:], lhsT=ones_sb[:], rhs=sqt[:],
                        start=(kd == 0), stop=(kd == KD - 1),
                    )
                lno = stat.tile([128, 512], F32, tag="lno")
                nc.scalar.activation(
                    lno[:], ssq[:], AF.Ln, bias=eps_sb[:], scale=1.0 / D
                )
                rstd = stat.tile([128, 512], F16, tag="rstd")
                nc.scalar.activation(rstd[:], lno[:], AF.Exp, scale=-0.5)
                for kd in range(KD):
                    sl = slice(kd * 2048 + qc * 512, kd * 2048 + qc * 512 + 512)
                    o = arp.tile([128, 512], F32, tag="u", bufs=2)
                    nc.vector.tensor_mul(o[:], hT_sb[:, sl], rstd[:])
                    nc.sync.dma_start(
                        out=out[kd * 128:(kd + 1) * 128,
                                (qc - QC // 2) * 512:(qc - QC // 2 + 1) * 512],
                        in_=o[:],
                    )

    _fix_drain_waits(nc)
    return nc


# ---------------- host side ----------------

def _rel_bucket_np(rel):
    """numpy replica of reference _rel_bucket (int32/float32 semantics)."""
    nb = NB // 2
    ret = (rel > 0).astype(np.int32) * nb
    arel = np.abs(rel)
    max_exact = nb // 2
    t = np.log(np.maximum(arel, 1).astype(np.float32) / np.float32(max_exact))
    t = t / np.float32(np.log(MAXD / max_exact)) * np.float32(nb - max_exact)
    large = max_exact + t.astype(np.int32)
    large = np.minimum(large, nb - 1)
    return ret + np.where(arel < max_exact, arel.astype(np.int32), large)


def _build_mt(rel_bias, core):
    """exp(bias) master table [HL, 128, 3968] for this core's heads."""
    d = np.arange(-(T - 1), T, dtype=np.int64)          # k - q in [-2047, 2047]
    buckets = _rel_bucket_np(d)                          # [4095]
    p = np.arange(128)[:, None]
    i = np.arange(3968)[None, :]
    idx = 3967 + p - i                                   # in [0, 4094]
    mts = []
    for hh in range(HL):
        head = core * HL + hh
        toep = rel_bias[buckets, head].astype(np.float32)  # [4095]
        mts.append(np.exp(toep)[idx])
    return np.stack(mts).astype(ml_dtypes.bfloat16)


_prog_cache = {}


def kernel(**inputs):
    input_ids = np.asarray(inputs["input_ids"]).astype(np.int64)
    memory = np.asarray(inputs["memory"], dtype=np.float32)
    embed = np.asarray(inputs["embed"], dtype=np.float32)
    Wq = np.asarray(inputs["Wq"], dtype=np.float32)
    Wk = np.asarray(inputs["Wk"], dtype=np.float32)
    Wv = np.asarray(inputs["Wv"], dtype=np.float32)
    Wo = np.asarray(inputs["Wo"], dtype=np.float32)
    ln1 = np.asarray(inputs["ln1"], dtype=np.float32)
    ln2 = np.asarray(inputs["ln2"], dtype=np.float32)
    wi0 = np.asarray(inputs["wi0"], dtype=np.float32)
    wi1 = np.asarray(inputs["wi1"], dtype=np.float32)
    wo_ff = np.asarray(inputs["wo_ff"], dtype=np.float32)
    final_ln = np.asarray(inputs["final_ln"], dtype=np.float32)
    rel_bias = np.asarray(inputs["rel_bias"], dtype=np.float32)

    bf = np.float16

    x = embed[input_ids[0]]                      # [S, D]
    h0 = np.concatenate([memory[0], x], axis=0)  # [T, D] f32
    h0T = np.ascontiguousarray(h0.T).astype(bf)  # [D, T] f16

    in_maps = []
    for c in range(NCORES):
        hs = slice(c * DH, (c + 1) * DH)
        fs = slice(c * FLR, (c + 1) * FLR)
        wq_c = (ln1[:, :, None] * Wq)[:, :, hs].astype(bf)          # [L, D, DH]
        wk_c = (ln1[:, :, None] * Wk)[:, :, hs].astype(bf)
        wv_c = (ln1[:, :, None] * Wv)[:, :, hs].astype(bf)
        wo_c = Wo[:, hs, :].astype(ml_dtypes.bfloat16)               # [L, DH, D]
        wi_c = np.zeros((L, D, 2 * FL), np.float32)
        wi_c[:, :, :FLR] = (ln2[:, :, None] * wi0)[:, :, fs]
        wi_c[:, :, FL:FL + FLR] = (ln2[:, :, None] * wi1)[:, :, fs]
        wf_c = np.zeros((L, FL, D), np.float32)
        wf_c[:, :FLR, :] = wo_ff[:, fs, :]
        in_maps.append({
            "h0T": h0T,
            "WQ": np.ascontiguousarray(wq_c),
            "WK": np.ascontiguousarray(wk_c),
            "WV": np.ascontiguousarray(wv_c),
            "WO": np.ascontiguousarray(wo_c),
            "WI": wi_c.astype(bf),
            "WF": wf_c.astype(bf),
            "MT": _build_mt(rel_bias, c),
        })

    if "nc" not in _prog_cache:
        _prog_cache["nc"] = build_program()
    nc = _prog_cache["nc"]
    _prog_cache["in_maps"] = in_maps

    res = run_bass_kernel_spmd(nc, in_maps, list(range(NCORES)))
    hidT = res.results[0]["out"]                 # [D, S] normalized, unweighted
    outp = hidT.T * final_ln[None, :] + memory[0]
    return outp[None].astype(np.float32)


if __name__ == "__main__":
    rng = np.random.default_rng(0)
    fake = {
        "input_ids": rng.integers(0, V, (B, S)),
        "memory": rng.standard_normal((B, S, D), dtype=np.float32),
        "embed": rng.standard_normal((V, D), dtype=np.float32) * 0.02,
        "Wq": rng.standard_normal((L, D, H * DK), dtype=np.float32) * 0.02,
        "Wk": rng.standard_normal((L, D, H * DK), dtype=np.float32) * 0.02,
        "Wv": rng.standard_normal((L, D, H * DK), dtype=np.float32) * 0.02,
        "Wo": rng.standard_normal((L, H * DK, D), dtype=np.float32) * 0.02,
        "ln1": np.ones((L, D), np.float32),
        "ln2": np.ones((L, D), np.float32),
        "wi0": rng.standard_normal((L, D, DFF), dtype=np.float32) * 0.02,
        "wi1": rng.standard_normal((L, D, DFF), dtype=np.float32) * 0.02,
        "wo_ff": rng.standard_normal((L, DFF, D), dtype=np.float32) * 0.02,
        "final_ln": np.ones((D,), np.float32),
        "rel_bias": rng.standard_normal((NB, H), dtype=np.float32) * 0.02,
    }
    o = kernel(**fake)
    print("out", o.shape, o.dtype, np.abs(o).mean())


# revision 37
# speedup vs baseline: 1.7115x; 1.4233x over previous
"""T5-style 4-layer encoder on 8 trn2 NeuronCores, tensor-parallel.

Sharding: heads (2/core) for attention, d_ff (352->384 padded /core) for FFN.
Two f16 AllReduces per layer (attn-out, ffn-out), each chunked in 2 so the
collective overlaps compute.

v2: feature-major residual stream. The residual h lives ONLY transposed
(hT, f16, [d, t]) so the per-layer PE transposes + PSUM evictions of the
old token-major scheme disappear. RMS stats come from sq(hT) reduced over
partitions by an all-ones matmul (result replicated across partitions),
rstd = exp(-0.5*ln(ssq/D+eps)) on ACT (ln+exp share one LUT set with the
attention exp, so no table reloads), and nT = hT * rstd is a 2x-mode DVE
multiply. AllReduce deltas return via transposed DMA (xbar) and are added
feature-major. The last layer only computes queries/FFN for the second
half of the sequence (the only half that reaches the output). Output
leaves feature-major; the host transposes.
"""
import os
import sys

sys.path.insert(0, "/opt/trn_rl_repo")

import numpy as np
import ml_dtypes

import concourse.bass as bass
import concourse.mybir as mybir
import concourse.tile as tile
from concourse.bass_utils import run_bass_kernel_spmd
from concourse.masks import make_identity

F32 = mybir.dt.float32
BF16 = mybir.dt.bfloat16
F16 = mybir.dt.float16
AF = mybir.ActivationFunctionType
OP = mybir.AluOpType

# model dims
L, D, H, DK, DFF, V = 4, 1024, 16, 64, 2816, 32128
B, S = 1, 1024
T = 2 * S
NB, MAXD = 32, 128
EPS = 1e-6

NCORES = 8
HL = H // NCORES          # 2 heads per core
DH = HL * DK              # 128 local head dims
FLR = DFF // NCORES       # 352 real local dff
FL = 384                  # padded local dff
F3 = FL // 128            # 3
TT = T // 128             # 16 token tiles
KD = D // 128             # 8 contraction tiles over D
QC = T // 512             # 4 query chunks
DC = D // 512             # 2 output-d chunks
RG = [list(range(NCORES))]

N_LAYERS = int(os.environ.get("KERN_LAYERS", str(L)))
CC_DTYPE = os.environ.get("KERN_CC_DTYPE", "f16")  # f16 | bf16 | f32
CC_DT = {"f16": F16, "bf16": BF16, "f32": F32}[CC_DTYPE]
CC_NP = {"f16": np.float16, "bf16": ml_dtypes.bfloat16, "f32": np.float32}[CC_DTYPE]
# last layer computes only second-half queries when running the full model
HALF_LAST = os.environ.get("KERN_HALF_LAST", "1") == "1" and N_LAYERS == L


def _fix_drain_waits(nc):
    """This walrus build has tight per-instruction sem-wait slot limits
    (Drain: none at all). Hoist excess waits onto preceding same-engine
    NoOps (<=2 each)."""
    for f in nc.m.functions:
        for bb in f.blocks:
            insts = bb.instructions
            i = 0
            while i < len(insts):
                ins = insts[i]
                si = ins.sync_info
                if si is None or not si.on_wait:
                    i += 1
                    continue
                keep = 0 if isinstance(ins, mybir.InstDrain) else 1
                waits = list(si.on_wait)
                if len(waits) <= keep:
                    i += 1
                    continue
                excess, kept = waits[: len(waits) - keep], waits[len(waits) - keep:]
                nops = [
                    mybir.InstNoOp(
                        name=f"{ins.name}_waitsplit{j}",
                        sync_info=mybir.SyncInfo(
                            on_wait=[w], on_update=[]
                        ),
                        bass_nofuse=True,
                        engine=ins.engine,
                    )
                    for j, w in enumerate(excess)
                ]
                ins.sync_info = mybir.SyncInfo(on_wait=kept, on_update=si.on_update)
                for k, nop in enumerate(nops):
                    insts.insert(i + k, nop)
                i += len(nops) + 1


def _layer_ranges(lyr):
    """(qc_lo, qc_hi, ti_lo, ti_hi) for this layer's query-side work."""
    if HALF_LAST and lyr == L - 1:
        return QC // 2, QC, TT // 2, TT
    return 0, QC, 0, TT


def build_program():
    nc = bass.Bass(num_devices=NCORES)

    h0T = nc.declare_dram_parameter("h0T", [D, T], F16, isOutput=False)
    WQ = nc.declare_dram_parameter("WQ", [L, D, DH], F16, isOutput=False)
    WK = nc.declare_dram_parameter("WK", [L, D, DH], F16, isOutput=False)
    WV = nc.declare_dram_parameter("WV", [L, D, DH], F16, isOutput=False)
    WO = nc.declare_dram_parameter("WO", [L, DH, D], BF16, isOutput=False)
    WI = nc.declare_dram_parameter("WI", [L, D, 2 * FL], F16, isOutput=False)
    WF = nc.declare_dram_parameter("WF", [L, FL, D], F16, isOutput=False)
    MT = nc.declare_dram_parameter("MT", [HL, 128, 3968], BF16, isOutput=False)
    out = nc.declare_dram_parameter("out", [D, S], F32, isOutput=True)

    # collective bounce buffers (plain dram tensors; pool tiles crash ENCD)
    ccin = {}
    ccout = {}
    ccrange = {}
    for name in ("attn", "ffn"):
        for lyr in range(N_LAYERS):
            _, _, ti_lo, ti_hi = _layer_ranges(lyr)
            half = (ti_hi - ti_lo) // 2
            for c in range(2):
                t0 = (ti_lo + c * half) * 128
                ntok = half * 128
                ccrange[(name, lyr, c)] = (t0, ntok)
                ccin[(name, lyr, c)] = nc.dram_tensor(
                    f"ccin_{name}_{lyr}_{c}", [ntok, D], CC_DT
                )
                ccout[(name, lyr, c)] = nc.dram_tensor(
                    f"ccout_{name}_{lyr}_{c}", [ntok, D], CC_DT, addr_space="Shared"
                )

    with tile.TileContext(nc) as tc:
        with (
            tc.tile_pool(name="pers", bufs=1) as pers,
            tc.tile_pool(name="wbuf", bufs=1) as wbuf,
            tc.tile_pool(name="stat", bufs=4) as stat,
            tc.tile_pool(name="sqp", bufs=4) as sqp,
            tc.tile_pool(name="ep", bufs=6) as ep,
            tc.tile_pool(name="arp", bufs=4) as arp,
            tc.tile_pool(name="resp", bufs=4) as resp,
            tc.tile_pool(name="ps_acc", bufs=3, space="PSUM") as ps_acc,
            tc.tile_pool(name="ps_tmp", bufs=4, space="PSUM") as ps_tmp,
            tc.tile_pool(name="ps_sm", bufs=1, space="PSUM") as ps_sm,
        ):
            # ---------- persistent SBUF ----------
            hT_sb = pers.tile([128, KD * 2048], F16)
            nT_sb = pers.tile([128, KD * 2048], F16)
            mt_sb = pers.tile([128, HL * 3968], BF16)
            ident = pers.tile([128, 128], F32)
            ones_sb = pers.tile([128, 128], F16)
            qT_sb = pers.tile([128, 2048], F16)
            kT_sb = pers.tile([128, 2048], F16)
            vaug_sb = pers.tile([128, TT * 130], BF16)
            oTu_sb = pers.tile([128, 2048], BF16)
            zrow_sb = [pers.tile([1, 2048], F32, name=f"zrow{i}") for i in range(HL)]
            rz_sb = pers.tile([128, TT * HL], F32)
            gT_sb = pers.tile([128, F3 * 2048], F16)

            wq_sb = wbuf.tile([128, KD * DH], F16)
            wk_sb = wbuf.tile([128, KD * DH], F16)
            wv_sb = wbuf.tile([128, KD * DH], F16)
            wo_sb = wbuf.tile([128, D], BF16)
            wi_sb = wbuf.tile([128, KD * 2 * FL], F16)
            wf_sb = wbuf.tile([128, F3 * D], F16)

            nc.vector.memset(ones_sb[:], 1.0)
            make_identity(nc, ident[:])
            eps_sb = pers.tile([128, 1], F32)
            nc.vector.memset(eps_sb[:], EPS)

            # load hT0 (feature-major), split per token-chunk so layer-0
            # rms1 starts after the first quarter lands
            for qc0 in range(QC):
                nc.sync.dma_start(
                    out=hT_sb[:].rearrange("p (kd t) -> p kd t", kd=KD)[
                        :, :, qc0 * 512:(qc0 + 1) * 512
                    ],
                    in_=h0T[:, qc0 * 512:(qc0 + 1) * 512].rearrange(
                        "(kd p) t -> p kd t", p=128
                    ),
                )
            nc.sync.dma_start(
                out=mt_sb[:].rearrange("p (hh c) -> p hh c", hh=HL),
                in_=MT[:].rearrange("hh p c -> p hh c"),
            )
            # ones columns of vaug (cols 64 and 129 of each 130-block)
            nc.vector.memset(
                vaug_sb[:].rearrange("p (ti j) -> p ti j", j=130)[:, :, 64:65], 1.0
            )
            nc.vector.memset(
                vaug_sb[:].rearrange("p (ti j) -> p ti j", j=130)[:, :, 129:130], 1.0
            )

            def rms_feature(dst, qc_lo, qc_hi):
                """hT (f16 feature-major) -> rms-normalized f16 nT chunks."""
                for qc in range(qc_lo, qc_hi):
                    ssq = ps_acc.tile([128, 512], F32, tag="pacc")
                    for kd in range(KD):
                        sl = slice(kd * 2048 + qc * 512, kd * 2048 + qc * 512 + 512)
                        sqt = sqp.tile([128, 512], F16, tag="sq")
                        nc.vector.tensor_mul(sqt[:], hT_sb[:, sl], hT_sb[:, sl])
                        nc.tensor.matmul(
                            ssq[:], lhsT=ones_sb[:], rhs=sqt[:],
                            start=(kd == 0), stop=(kd == KD - 1),
                        )
                    # rstd = 1/sqrt(ssq/D + eps) = exp(-0.5*ln(ssq/D + eps));
                    # ln+exp share one ACT table set with the attention exp,
                    # so no LUT reloads (sqrt lives in a different set)
                    lno = stat.tile([128, 512], F32, tag="lno")
                    nc.scalar.activation(
                        lno[:], ssq[:], AF.Ln, bias=eps_sb[:], scale=1.0 / D
                    )
                    rstd = stat.tile([128, 512], F16, tag="rstd")
                    nc.scalar.activation(rstd[:], lno[:], AF.Exp, scale=-0.5)
                    for kd in range(KD):
                        sl = slice(kd * 2048 + qc * 512, kd * 2048 + qc * 512 + 512)
                        nc.vector.tensor_mul(dst[:, sl], hT_sb[:, sl], rstd[:])

            def do_allreduce(name, lyr, c):
                """AllReduce chunk c + transposed residual add into hT."""
                if os.environ.get("KERN_NO_CC") == "1":
                    nc.sync.dma_start(
                        out=ccout[(name, lyr, c)][:], in_=ccin[(name, lyr, c)][:]
                    )
                else:
                    nc.gpsimd.collective_compute(
                        "AllReduce", OP.add, replica_groups=RG,
                        ins=[ccin[(name, lyr, c)][:].opt()],
                        outs=[ccout[(name, lyr, c)][:].opt()],
                    )
                t0, ntok = ccrange[(name, lyr, c)]
                for kd in range(KD):
                    dstg = resp.tile([128, 1024], CC_DT, tag="rstg")
                    nc.sync.dma_start_transpose(
                        dstg[:, 0:ntok], ccout[(name, lyr, c)][:, kd * 128:(kd + 1) * 128]
                    )
                    sl = slice(kd * 2048 + t0, kd * 2048 + t0 + ntok)
                    nc.vector.tensor_add(hT_sb[:, sl], hT_sb[:, sl], dstg[:, 0:ntok])

            for lyr in range(N_LAYERS):
                qc_lo, qc_hi, ti_lo, ti_hi = _layer_ranges(lyr)
                # ---------- load layer weights ----------
                nc.sync.dma_start(
                    out=wq_sb[:].rearrange("p (kd m) -> p kd m", kd=KD),
                    in_=WQ[lyr].rearrange("(kd p) m -> p kd m", p=128),
                )
                nc.sync.dma_start(
                    out=wk_sb[:].rearrange("p (kd m) -> p kd m", kd=KD),
                    in_=WK[lyr].rearrange("(kd p) m -> p kd m", p=128),
                )
                nc.sync.dma_start(
                    out=wv_sb[:].rearrange("p (kd m) -> p kd m", kd=KD),
                    in_=WV[lyr].rearrange("(kd p) m -> p kd m", p=128),
                )
                nc.sync.dma_start(out=wo_sb[:], in_=WO[lyr])
                nc.sync.dma_start(
                    out=wi_sb[:].rearrange("p (kd m) -> p kd m", kd=KD),
                    in_=WI[lyr].rearrange("(kd p) m -> p kd m", p=128),
                )
                nc.sync.dma_start(
                    out=wf_sb[:].rearrange("p (f3 m) -> p f3 m", f3=F3),
                    in_=WF[lyr].rearrange("(f3 p) m -> p f3 m", p=128),
                )

                # ---------- rms1 (full: K/V need all tokens) ----------
                rms_feature(nT_sb, 0, QC)

                # ---------- q/k projections (feature-major out) ----------
                for w_sb, dst, lo, hi in (
                    (wq_sb, qT_sb, qc_lo, qc_hi), (wk_sb, kT_sb, 0, QC)
                ):
                    for qc in range(lo, hi):
                        pq = ps_acc.tile([128, 512], F32, tag="pacc")
                        for kd in range(KD):
                            nc.tensor.matmul(
                                pq[:],
                                lhsT=w_sb[:, kd * DH:(kd + 1) * DH],
                                rhs=nT_sb[:, kd * 2048 + qc * 512: kd * 2048 + qc * 512 + 512],
                                start=(kd == 0), stop=(kd == KD - 1),
                            )
                        nc.scalar.copy(dst[:, qc * 512:(qc + 1) * 512], pq[:])

                # ---------- v projection (token-major into vaug) ----------
                for ti in range(TT):
                    pv = ps_sm.tile([128, 128], F32, tag="pt")
                    for kd in range(KD):
                        nc.tensor.matmul(
                            pv[:],
                            lhsT=nT_sb[:, kd * 2048 + ti * 128: kd * 2048 + (ti + 1) * 128],
                            rhs=wv_sb[:, kd * DH:(kd + 1) * DH],
                            start=(kd == 0), stop=(kd == KD - 1),
                        )
                    nc.scalar.copy(
                        vaug_sb[:, ti * 130: ti * 130 + 130]
                        .rearrange("p (j k) -> p j k", j=2, k=65)[:, :, 0:64],
                        pv[:].rearrange("p (j k) -> p j k", j=2, k=64),
                    )

                # ---------- attention (transposed scores; per local head) ----------
                for hh in range(HL):
                    mtb = hh * 3968
                    for qc in range(qc_lo, qc_hi):
                        po = ps_acc.tile([128, 512], F32, tag="pacc")
                        for kt in range(TT):
                            ps = ps_tmp.tile([128, 512], F32, tag="ptmp")
                            nc.tensor.matmul(
                                ps[:],
                                lhsT=kT_sb[hh * 64:(hh + 1) * 64, kt * 128:(kt + 1) * 128],
                                rhs=qT_sb[hh * 64:(hh + 1) * 64, qc * 512:(qc + 1) * 512],
                                start=True, stop=True,
                            )
                            e1 = ep.tile([128, 512], BF16, tag="e1")
                            nc.scalar.activation(e1[:], ps[:], AF.Exp)
                            off = mtb + qc * 512 - kt * 128 + 1920
                            nc.vector.tensor_mul(e1[:], e1[:], mt_sb[:, off: off + 512])
                            nc.tensor.matmul(
                                po[0:65, :],
                                lhsT=vaug_sb[:, kt * 130 + hh * 65: kt * 130 + hh * 65 + 65],
                                rhs=e1[:],
                                start=(kt == 0), stop=(kt == TT - 1),
                                skip_group_check=True,
                            )
                        nc.scalar.copy(
                            oTu_sb[hh * 64:(hh + 1) * 64, qc * 512:(qc + 1) * 512],
                            po[0:64, :],
                        )
                        nc.scalar.copy(
                            zrow_sb[hh][:, qc * 512:(qc + 1) * 512], po[64:65, :]
                        )

                # ---------- Z transpose + reciprocal ----------
                for ti in range(ti_lo, ti_hi):
                    pz = ps_sm.tile([128, 128], F32, tag="pt")
                    for hh in range(HL):
                        nc.tensor.matmul(
                            pz[0:128, hh:hh + 1],
                            lhsT=zrow_sb[hh][0:1, ti * 128:(ti + 1) * 128],
                            rhs=ident[0:1, 0:1],
                            start=True, stop=True,
                            skip_group_check=True,
                        )
                    nc.vector.reciprocal(
                        rz_sb[:, ti * HL:(ti + 1) * HL], pz[0:128, 0:HL]
                    )

                # ---------- Wo (+1/Z fold) -> attn partial -> AR ----------
                half = (ti_hi - ti_lo) // 2
                for c in range(2):
                    tis = range(ti_lo + c * half, ti_lo + (c + 1) * half)
                    for ti in tis:
                        rzA = rz_sb[:, ti * HL: ti * HL + 1]
                        rzB = rz_sb[:, ti * HL + 1: ti * HL + 2]
                        for dc in range(DC):
                            pA = ps_tmp.tile([128, 512], F32, tag="ptmp")
                            pB = ps_tmp.tile([128, 512], F32, tag="ptmp")
                            nc.tensor.matmul(
                                pA[:],
                                lhsT=oTu_sb[0:64, ti * 128:(ti + 1) * 128],
                                rhs=wo_sb[0:64, dc * 512:(dc + 1) * 512],
                                start=True, stop=True,
                            )
                            nc.tensor.matmul(
                                pB[:],
                                lhsT=oTu_sb[64:128, ti * 128:(ti + 1) * 128],
                                rhs=wo_sb[64:128, dc * 512:(dc + 1) * 512],
                                start=True, stop=True,
                            )
                            u = arp.tile([128, 512], F32, tag="u", bufs=2)
                            nc.vector.tensor_scalar_mul(u[:], pB[:], rzB)
                            a = arp.tile([128, 512], CC_DT, tag="a")
                            nc.vector.scalar_tensor_tensor(
                                a[:], in0=pA[:], scalar=rzA, in1=u[:],
                                op0=OP.mult, op1=OP.add,
                            )
                            nc.sync.dma_start(
                                out=ccin[("attn", lyr, c)][
                                    (ti - ti_lo - c * half) * 128:
                                    (ti - ti_lo - c * half + 1) * 128,
                                    dc * 512:(dc + 1) * 512,
                                ],
                                in_=a[:],
                            )
                    do_allreduce("attn", lyr, c)

                # ---------- rms2 (query range only) ----------
                rms_feature(nT_sb, qc_lo, qc_hi)

                # ---------- FFN up (wi0|wi1), gelu*gate -> gT (feature-major) ----------
                for f3 in range(F3):
                    for qc in range(qc_lo, qc_hi):
                        pg0 = ps_acc.tile([128, 512], F32, tag="pacc")
                        pg1 = ps_tmp.tile([128, 512], F32, tag="ptmp")
                        for kd in range(KD):
                            rhs = nT_sb[:, kd * 2048 + qc * 512: kd * 2048 + qc * 512 + 512]
                            nc.tensor.matmul(
                                pg0[:],
                                lhsT=wi_sb[:, kd * 2 * FL + f3 * 128: kd * 2 * FL + (f3 + 1) * 128],
                                rhs=rhs,
                                start=(kd == 0), stop=(kd == KD - 1),
                                skip_group_check=True,
                            )
                            nc.tensor.matmul(
                                pg1[:],
                                lhsT=wi_sb[:, kd * 2 * FL + FL + f3 * 128: kd * 2 * FL + FL + (f3 + 1) * 128],
                                rhs=rhs,
                                start=(kd == 0), stop=(kd == KD - 1),
                                skip_group_check=True,
                            )
                        gt = ep.tile([128, 512], F32, tag="gt", bufs=2)
                        nc.scalar.activation(gt[:], pg0[:], AF.Gelu_apprx_tanh)
                        nc.vector.tensor_mul(
                            gT_sb[:, f3 * 2048 + qc * 512: f3 * 2048 + (qc + 1) * 512],
                            gt[:], pg1[:],
                        )

                # ---------- FFN down -> partial -> AR ----------
                for c in range(2):
                    tis = range(ti_lo + c * half, ti_lo + (c + 1) * half)
                    for i, ti in enumerate(tis):
                        for dc in range(DC):
                            pf = ps_tmp.tile([128, 512], F32, tag="ptmp")
                            for f3 in range(F3):
                                nc.tensor.matmul(
                                    pf[:],
                                    lhsT=gT_sb[:, f3 * 2048 + ti * 128: f3 * 2048 + (ti + 1) * 128],
                                    rhs=wf_sb[:, f3 * 1024 + dc * 512: f3 * 1024 + (dc + 1) * 512],
                                    start=(f3 == 0), stop=(f3 == F3 - 1),
                                )
                            a = arp.tile([128, 512], CC_DT, tag="a")
                            nc.scalar.copy(a[:], pf[:])
                            nc.sync.dma_start(
                                out=ccin[("ffn", lyr, c)][
                                    i * 128:(i + 1) * 128,
                                    dc * 512:(dc + 1) * 512,
                                ],
                                in_=a[:],
                            )
                    do_allreduce("ffn", lyr, c)

            # ---------- final rms on second half, output (feature-major) ----------
            for qc in range(QC // 2, QC):
                ssq = ps_acc.tile([128, 512], F32, tag="pacc")
                for kd in range(KD):
                    sl = slice(kd * 2048 + qc * 512, kd * 2048 + qc * 512 + 512)
                    sqt = sqp.tile([128, 512], F16, tag="sq")
                    nc.vector.tensor_mul(sqt[:], hT_sb[:, sl], hT_sb[:, sl])
                    nc.tensor.matmul(
                        ssq[:], lhsT=ones_sb[:], rhs=sqt[:],
                        start=(kd == 0), stop=(kd == KD - 1),
                    )
                lno = stat.tile([128, 512], F32, tag="lno")
                nc.scalar.activation(
                    lno[:], ssq[:], AF.Ln, bias=eps_sb[:], scale=1.0 / D
                )
                rstd = stat.tile([128, 512], F16, tag="rstd")
                nc.scalar.activation(rstd[:], lno[:], AF.Exp, scale=-0.5)
                for kd in range(KD):
                    sl = slice(kd * 2048 + qc * 512, kd * 2048 + qc * 512 + 512)
                    o = arp.tile([128, 512], F32, tag="u", bufs=2)
                    nc.vector.tensor_mul(o[:], hT_sb[:, sl], rstd[:])
                    nc.sync.dma_start(
                        out=out[kd * 128:(kd + 1) * 128,
                                (qc - QC // 2) * 512:(qc - QC // 2 + 1) * 512],
                        in_=o[:],
                    )

    _fix_drain_waits(nc)
    return nc


# ---------------- host side ----------------

def _rel_bucket_np(rel):
    """numpy replica of reference _rel_bucket (int32/float32 semantics)."""
    nb = NB // 2
    ret = (rel > 0).astype(np.int32) * nb
    arel = np.abs(rel)
    max_exact = nb // 2
    t = np.log(np.maximum(arel, 1).astype(np.float32) / np.float32(max_exact))
    t = t / np.float32(np.log(MAXD / max_exact)) * np.float32(nb - max_exact)
    large = max_exact + t.astype(np.int32)
    large = np.minimum(large, nb - 1)
    return ret + np.where(arel < max_exact, arel.astype(np.int32), large)


def _build_mt(rel_bias, core):
    """exp(bias) master table [HL, 128, 3968] for this core's heads."""
    d = np.arange(-(T - 1), T, dtype=np.int64)          # k - q in [-2047, 2047]
    buckets = _rel_bucket_np(d)                          # [4095]
    p = np.arange(128)[:, None]
    i = np.arange(3968)[None, :]
    idx = 3967 + p - i                                   # in [0, 4094]
    mts = []
    for hh in range(HL):
        head = core * HL + hh
        toep = rel_bias[buckets, head].astype(np.float32)  # [4095]
        mts.append(np.exp(toep)[idx])
    return np.stack(mts).astype(ml_dtypes.bfloat16)


_prog_cache = {}


def kernel(**inputs):
    input_ids = np.asarray(inputs["input_ids"]).astype(np.int64)
    memory = np.asarray(inputs["memory"], dtype=np.float32)
    embed = np.asarray(inputs["embed"], dtype=np.float32)
    Wq = np.asarray(inputs["Wq"], dtype=np.float32)
    Wk = np.asarray(inputs["Wk"], dtype=np.float32)
    Wv = np.asarray(inputs["Wv"], dtype=np.float32)
    Wo = np.asarray(inputs["Wo"], dtype=np.float32)
    ln1 = np.asarray(inputs["ln1"], dtype=np.float32)
    ln2 = np.asarray(inputs["ln2"], dtype=np.float32)
    wi0 = np.asarray(inputs["wi0"], dtype=np.float32)
    wi1 = np.asarray(inputs["wi1"], dtype=np.float32)
    wo_ff = np.asarray(inputs["wo_ff"], dtype=np.float32)
    final_ln = np.asarray(inputs["final_ln"], dtype=np.float32)
    rel_bias = np.asarray(inputs["rel_bias"], dtype=np.float32)

    bf = np.float16

    x = embed[input_ids[0]]                      # [S, D]
    h0 = np.concatenate([memory[0], x], axis=0)  # [T, D] f32
    h0T = np.ascontiguousarray(h0.T).astype(bf)  # [D, T] f16

    in_maps = []
    for c in range(NCORES):
        hs = slice(c * DH, (c + 1) * DH)
        fs = slice(c * FLR, (c + 1) * FLR)
        wq_c = (ln1[:, :, None] * Wq)[:, :, hs].astype(bf)          # [L, D, DH]
        wk_c = (ln1[:, :, None] * Wk)[:, :, hs].astype(bf)
        wv_c = (ln1[:, :, None] * Wv)[:, :, hs].astype(bf)
        wo_c = Wo[:, hs, :].astype(ml_dtypes.bfloat16)               # [L, DH, D]
        wi_c = np.zeros((L, D, 2 * FL), np.float32)
        wi_c[:, :, :FLR] = (ln2[:, :, None] * wi0)[:, :, fs]
        wi_c[:, :, FL:FL + FLR] = (ln2[:, :, None] * wi1)[:, :, fs]
        wf_c = np.zeros((L, FL, D), np.float32)
        wf_c[:, :FLR, :] = wo_ff[:, fs, :]
        in_maps.append({
            "h0T": h0T,
            "WQ": np.ascontiguousarray(wq_c),
            "WK": np.ascontiguousarray(wk_c),
            "WV": np.ascontiguousarray(wv_c),
            "WO": np.ascontiguousarray(wo_c),
            "WI": wi_c.astype(bf),
            "WF": wf_c.astype(bf),
            "MT": _build_mt(rel_bias, c),
        })

    if "nc" not in _prog_cache:
        _prog_cache["nc"] = build_program()
    nc = _prog_cache["nc"]
    _prog_cache["in_maps"] = in_maps

    res = run_bass_kernel_spmd(nc, in_maps, list(range(NCORES)))
    hidT = res.results[0]["out"]                 # [D, S] normalized, unweighted
    outp = hidT.T * final_ln[None, :] + memory[0]
    return outp[None].astype(np.float32)


if __name__ == "__main__":
    rng = np.random.default_rng(0)
    fake = {
        "input_ids": rng.integers(0, V, (B, S)),
        "memory": rng.standard_normal((B, S, D), dtype=np.float32),
        "embed": rng.standard_normal((V, D), dtype=np.float32) * 0.02,
        "Wq": rng.standard_normal((L, D, H * DK), dtype=np.float32) * 0.02,
        "Wk": rng.standard_normal((L, D, H * DK), dtype=np.float32) * 0.02,
        "Wv": rng.standard_normal((L, D, H * DK), dtype=np.float32) * 0.02,
        "Wo": rng.standard_normal((L, H * DK, D), dtype=np.float32) * 0.02,
        "ln1": np.ones((L, D), np.float32),
        "ln2": np.ones((L, D), np.float32),
        "wi0": rng.standard_normal((L, D, DFF), dtype=np.float32) * 0.02,
        "wi1": rng.standard_normal((L, D, DFF), dtype=np.float32) * 0.02,
        "wo_ff": rng.standard_normal((L, DFF, D), dtype=np.float32) * 0.02,
        "final_ln": np.ones((D,), np.float32),
        "rel_bias": rng.standard_normal((NB, H), dtype=np.float32) * 0.02,
    }
    o = kernel(**fake)
    print("out", o.shape, o.dtype, np.abs(o).mean())


# revision 38
# speedup vs baseline: 5.1772x; 3.0249x over previous
"""T5-style 4-layer encoder on 8 trn2 NeuronCores, tensor-parallel.

Sharding: heads (2/core) for attention, d_ff (352->384 padded /core) for FFN.
Two f16 AllReduces per layer (attn-out, ffn-out), each chunked in 2 so the
collective overlaps compute.

v2: feature-major residual stream. The residual h lives ONLY transposed
(hT, f16, [d, t]) so the per-layer PE transposes + PSUM evictions of the
old token-major scheme disappear. RMS stats come from sq(hT) reduced over
partitions by an all-ones matmul (result replicated across partitions),
rstd = exp(-0.5*ln(ssq/D+eps)) on ACT (ln+exp share one LUT set with the
attention exp, so no table reloads), and nT = hT * rstd is a 2x-mode DVE
multiply. AllReduce deltas return via transposed DMA (xbar) and are added
feature-major. The last layer only computes queries/FFN for the second
half of the sequence (the only half that reaches the output). Output
leaves feature-major; the host transposes.
"""
import os
import sys

sys.path.insert(0, "/opt/trn_rl_repo")

import numpy as np
import ml_dtypes

import concourse.bass as bass
import concourse.mybir as mybir
import concourse.tile as tile
from concourse.bass_utils import run_bass_kernel_spmd
from concourse.masks import make_identity

F32 = mybir.dt.float32
BF16 = mybir.dt.bfloat16
F16 = mybir.dt.float16
AF = mybir.ActivationFunctionType
OP = mybir.AluOpType

# model dims
L, D, H, DK, DFF, V = 4, 1024, 16, 64, 2816, 32128
B, S = 1, 1024
T = 2 * S
NB, MAXD = 32, 128
EPS = 1e-6

NCORES = 8
HL = H // NCORES          # 2 heads per core
DH = HL * DK              # 128 local head dims
FLR = DFF // NCORES       # 352 real local dff
FL = 384                  # padded local dff
F3 = FL // 128            # 3
TT = T // 128             # 16 token tiles
KD = D // 128             # 8 contraction tiles over D
QC = T // 512             # 4 query chunks
DC = D // 512             # 2 output-d chunks
RG = [list(range(NCORES))]

N_LAYERS = int(os.environ.get("KERN_LAYERS", str(L)))
CC_DTYPE = os.environ.get("KERN_CC_DTYPE", "f16")  # f16 | bf16 | f32
CC_DT = {"f16": F16, "bf16": BF16, "f32": F32}[CC_DTYPE]
CC_NP = {"f16": np.float16, "bf16": ml_dtypes.bfloat16, "f32": np.float32}[CC_DTYPE]
# last layer computes only second-half queries when running the full model
HALF_LAST = os.environ.get("KERN_HALF_LAST", "1") == "1" and N_LAYERS == L


def _fix_drain_waits(nc):
    """This walrus build has tight per-instruction sem-wait slot limits
    (Drain: none at all). Hoist excess waits onto preceding same-engine
    NoOps (<=2 each)."""
    for f in nc.m.functions:
        for bb in f.blocks:
            insts = bb.instructions
            i = 0
            while i < len(insts):
                ins = insts[i]
                si = ins.sync_info
                if si is None or not si.on_wait:
                    i += 1
                    continue
                keep = 0 if isinstance(ins, mybir.InstDrain) else 1
                waits = list(si.on_wait)
                if len(waits) <= keep:
                    i += 1
                    continue
                excess, kept = waits[: len(waits) - keep], waits[len(waits) - keep:]
                nops = [
                    mybir.InstNoOp(
                        name=f"{ins.name}_waitsplit{j}",
                        sync_info=mybir.SyncInfo(
                            on_wait=[w], on_update=[]
                        ),
                        bass_nofuse=True,
                        engine=ins.engine,
                    )
                    for j, w in enumerate(excess)
                ]
                ins.sync_info = mybir.SyncInfo(on_wait=kept, on_update=si.on_update)
                for k, nop in enumerate(nops):
                    insts.insert(i + k, nop)
                i += len(nops) + 1


def _layer_ranges(lyr):
    """(qc_lo, qc_hi, ti_lo, ti_hi) for this layer's query-side work."""
    if HALF_LAST and lyr == L - 1:
        return QC // 2, QC, TT // 2, TT
    return 0, QC, 0, TT


def build_program():
    nc = bass.Bass(num_devices=NCORES)

    h0T = nc.declare_dram_parameter("h0T", [D, T], F16, isOutput=False)
    WQ = nc.declare_dram_parameter("WQ", [L, D, DH], F16, isOutput=False)
    WK = nc.declare_dram_parameter("WK", [L, D, DH], F16, isOutput=False)
    WV = nc.declare_dram_parameter("WV", [L, D, DH], F16, isOutput=False)
    WO = nc.declare_dram_parameter("WO", [L, DH, D], BF16, isOutput=False)
    WI = nc.declare_dram_parameter("WI", [L, D, 2 * FL], F16, isOutput=False)
    WF = nc.declare_dram_parameter("WF", [L, FL, D], F16, isOutput=False)
    MT = nc.declare_dram_parameter("MT", [HL, 128, 3968], BF16, isOutput=False)
    out = nc.declare_dram_parameter("out", [D, S], F32, isOutput=True)

    # collective bounce buffers (plain dram tensors; pool tiles crash ENCD)
    ccin = {}
    ccout = {}
    ccrange = {}
    for name in ("attn", "ffn"):
        for lyr in range(N_LAYERS):
            _, _, ti_lo, ti_hi = _layer_ranges(lyr)
            half = (ti_hi - ti_lo) // 2
            for c in range(2):
                t0 = (ti_lo + c * half) * 128
                ntok = half * 128
                ccrange[(name, lyr, c)] = (t0, ntok)
                ccin[(name, lyr, c)] = nc.dram_tensor(
                    f"ccin_{name}_{lyr}_{c}", [ntok, D], CC_DT
                )
                ccout[(name, lyr, c)] = nc.dram_tensor(
                    f"ccout_{name}_{lyr}_{c}", [ntok, D], CC_DT, addr_space="Shared"
                )

    with tile.TileContext(nc) as tc:
        with (
            tc.tile_pool(name="pers", bufs=1) as pers,
            tc.tile_pool(name="wbuf", bufs=1) as wbuf,
            tc.tile_pool(name="stat", bufs=6) as stat,
            tc.tile_pool(name="sqp", bufs=4) as sqp,
            tc.tile_pool(name="ep", bufs=6) as ep,
            tc.tile_pool(name="arp", bufs=5) as arp,
            tc.tile_pool(name="resp", bufs=6) as resp,
            tc.tile_pool(name="ps_acc", bufs=3, space="PSUM") as ps_acc,
            tc.tile_pool(name="ps_tmp", bufs=4, space="PSUM") as ps_tmp,
            tc.tile_pool(name="ps_sm", bufs=1, space="PSUM") as ps_sm,
        ):
            # ---------- persistent SBUF ----------
            hT_sb = pers.tile([128, KD * 2048], F16)
            nT_sb = pers.tile([128, KD * 2048], F16)
            mt_sb = pers.tile([128, HL * 3968], BF16)
            ident = pers.tile([128, 128], F32)
            ones_sb = pers.tile([128, 128], F16)
            qT_sb = pers.tile([128, 2048], F16)
            kT_sb = pers.tile([128, 2048], F16)
            vaug_sb = pers.tile([128, TT * 130], BF16)
            oTu_sb = pers.tile([128, 2048], BF16)
            zrow_sb = [pers.tile([1, 2048], F32, name=f"zrow{i}") for i in range(HL)]
            rz_sb = pers.tile([128, TT * HL], F32)
            gT_sb = pers.tile([128, F3 * 2048], F16)

            wq_sb = wbuf.tile([128, KD * DH], F16)
            wk_sb = wbuf.tile([128, KD * DH], F16)
            wv_sb = wbuf.tile([128, KD * DH], F16)
            wo_sb = wbuf.tile([128, D], BF16)
            wi_sb = wbuf.tile([128, KD * 2 * FL], F16)
            wf_sb = wbuf.tile([128, F3 * D], F16)

            nc.vector.memset(ones_sb[:], 1.0)
            make_identity(nc, ident[:])
            eps_sb = pers.tile([128, 1], F32)
            nc.vector.memset(eps_sb[:], EPS)

            # load hT0 (feature-major), split per token-chunk so layer-0
            # rms1 starts after the first quarter lands
            for qc0 in range(QC):
                nc.sync.dma_start(
                    out=hT_sb[:].rearrange("p (kd t) -> p kd t", kd=KD)[
                        :, :, qc0 * 512:(qc0 + 1) * 512
                    ],
                    in_=h0T[:, qc0 * 512:(qc0 + 1) * 512].rearrange(
                        "(kd p) t -> p kd t", p=128
                    ),
                )
            nc.sync.dma_start(
                out=mt_sb[:].rearrange("p (hh c) -> p hh c", hh=HL),
                in_=MT[:].rearrange("hh p c -> p hh c"),
            )
            # ones columns of vaug (cols 64 and 129 of each 130-block)
            nc.vector.memset(
                vaug_sb[:].rearrange("p (ti j) -> p ti j", j=130)[:, :, 64:65], 1.0
            )
            nc.vector.memset(
                vaug_sb[:].rearrange("p (ti j) -> p ti j", j=130)[:, :, 129:130], 1.0
            )

            def rms_feature(dst, qc_lo, qc_hi):
                """hT (f16 feature-major) -> rms-normalized f16 nT chunks."""
                for qc in range(qc_lo, qc_hi):
                    ssq = ps_acc.tile([128, 512], F32, tag="pacc")
                    for kd in range(KD):
                        sl = slice(kd * 2048 + qc * 512, kd * 2048 + qc * 512 + 512)
                        sqt = sqp.tile([128, 512], F16, tag="sq")
                        nc.vector.tensor_mul(sqt[:], hT_sb[:, sl], hT_sb[:, sl])
                        nc.tensor.matmul(
                            ssq[:], lhsT=ones_sb[:], rhs=sqt[:],
                            start=(kd == 0), stop=(kd == KD - 1),
                        )
                    # rstd = 1/sqrt(ssq/D + eps) = exp(-0.5*ln(ssq/D + eps));
                    # ln+exp share one ACT table set with the attention exp,
                    # so no LUT reloads (sqrt lives in a different set)
                    lno = stat.tile([128, 512], F32, tag="lno")
                    nc.scalar.activation(
                        lno[:], ssq[:], AF.Ln, bias=eps_sb[:], scale=1.0 / D
                    )
                    rstd = stat.tile([128, 512], F16, tag="rstd")
                    nc.scalar.activation(rstd[:], lno[:], AF.Exp, scale=-0.5)
                    for kd in range(KD):
                        sl = slice(kd * 2048 + qc * 512, kd * 2048 + qc * 512 + 512)
                        nc.vector.tensor_mul(dst[:, sl], hT_sb[:, sl], rstd[:])

            def do_allreduce(name, lyr, c):
                """AllReduce chunk c + transposed residual add into hT."""
                if os.environ.get("KERN_NO_CC") == "1":
                    nc.sync.dma_start(
                        out=ccout[(name, lyr, c)][:], in_=ccin[(name, lyr, c)][:]
                    )
                else:
                    nc.gpsimd.collective_compute(
                        "AllReduce", OP.add, replica_groups=RG,
                        ins=[ccin[(name, lyr, c)][:].opt()],
                        outs=[ccout[(name, lyr, c)][:].opt()],
                    )
                t0, ntok = ccrange[(name, lyr, c)]
                for kd in range(KD):
                    dstg = resp.tile([128, 1024], CC_DT, tag="rstg")
                    nc.sync.dma_start_transpose(
                        dstg[:, 0:ntok], ccout[(name, lyr, c)][:, kd * 128:(kd + 1) * 128]
                    )
                    sl = slice(kd * 2048 + t0, kd * 2048 + t0 + ntok)
                    nc.vector.tensor_add(hT_sb[:, sl], hT_sb[:, sl], dstg[:, 0:ntok])

            for lyr in range(N_LAYERS):
                qc_lo, qc_hi, ti_lo, ti_hi = _layer_ranges(lyr)
                # ---------- load layer weights ----------
                nc.sync.dma_start(
                    out=wq_sb[:].rearrange("p (kd m) -> p kd m", kd=KD),
                    in_=WQ[lyr].rearrange("(kd p) m -> p kd m", p=128),
                )
                nc.sync.dma_start(
                    out=wk_sb[:].rearrange("p (kd m) -> p kd m", kd=KD),
                    in_=WK[lyr].rearrange("(kd p) m -> p kd m", p=128),
                )
                nc.sync.dma_start(
                    out=wv_sb[:].rearrange("p (kd m) -> p kd m", kd=KD),
                    in_=WV[lyr].rearrange("(kd p) m -> p kd m", p=128),
                )
                nc.sync.dma_start(out=wo_sb[:], in_=WO[lyr])
                nc.sync.dma_start(
                    out=wi_sb[:].rearrange("p (kd m) -> p kd m", kd=KD),
                    in_=WI[lyr].rearrange("(kd p) m -> p kd m", p=128),
                )
                nc.sync.dma_start(
                    out=wf_sb[:].rearrange("p (f3 m) -> p f3 m", f3=F3),
                    in_=WF[lyr].rearrange("(f3 p) m -> p f3 m", p=128),
                )

                # ---------- rms1 (full: K/V need all tokens) ----------
                rms_feature(nT_sb, 0, QC)

                # ---------- q/k projections (feature-major out) ----------
                for w_sb, dst, lo, hi in (
                    (wq_sb, qT_sb, qc_lo, qc_hi), (wk_sb, kT_sb, 0, QC)
                ):
                    for qc in range(lo, hi):
                        pq = ps_acc.tile([128, 512], F32, tag="pacc")
                        for kd in range(KD):
                            nc.tensor.matmul(
                                pq[:],
                                lhsT=w_sb[:, kd * DH:(kd + 1) * DH],
                                rhs=nT_sb[:, kd * 2048 + qc * 512: kd * 2048 + qc * 512 + 512],
                                start=(kd == 0), stop=(kd == KD - 1),
                            )
                        nc.scalar.copy(dst[:, qc * 512:(qc + 1) * 512], pq[:])

                # ---------- v projection (token-major into vaug) ----------
                for ti in range(TT):
                    pv = ps_sm.tile([128, 128], F32, tag="pt")
                    for kd in range(KD):
                        nc.tensor.matmul(
                            pv[:],
                            lhsT=nT_sb[:, kd * 2048 + ti * 128: kd * 2048 + (ti + 1) * 128],
                            rhs=wv_sb[:, kd * DH:(kd + 1) * DH],
                            start=(kd == 0), stop=(kd == KD - 1),
                        )
                    nc.scalar.copy(
                        vaug_sb[:, ti * 130: ti * 130 + 130]
                        .rearrange("p (j k) -> p j k", j=2, k=65)[:, :, 0:64],
                        pv[:].rearrange("p (j k) -> p j k", j=2, k=64),
                    )

                # ---------- attention (transposed scores; per local head) ----------
                for hh in range(HL):
                    mtb = hh * 3968
                    for qc in range(qc_lo, qc_hi):
                        po = ps_acc.tile([128, 512], F32, tag="pacc")
                        for kt in range(TT):
                            ps = ps_tmp.tile([128, 512], F32, tag="ptmp")
                            nc.tensor.matmul(
                                ps[:],
                                lhsT=kT_sb[hh * 64:(hh + 1) * 64, kt * 128:(kt + 1) * 128],
                                rhs=qT_sb[hh * 64:(hh + 1) * 64, qc * 512:(qc + 1) * 512],
                                start=True, stop=True,
                            )
                            e1 = ep.tile([128, 512], BF16, tag="e1")
                            nc.scalar.activation(e1[:], ps[:], AF.Exp)
                            off = mtb + qc * 512 - kt * 128 + 1920
                            nc.vector.tensor_mul(e1[:], e1[:], mt_sb[:, off: off + 512])
                            nc.tensor.matmul(
                                po[0:65, :],
                                lhsT=vaug_sb[:, kt * 130 + hh * 65: kt * 130 + hh * 65 + 65],
                                rhs=e1[:],
                                start=(kt == 0), stop=(kt == TT - 1),
                                skip_group_check=True,
                            )
                        nc.scalar.copy(
                            oTu_sb[hh * 64:(hh + 1) * 64, qc * 512:(qc + 1) * 512],
                            po[0:64, :],
                        )
                        nc.scalar.copy(
                            zrow_sb[hh][:, qc * 512:(qc + 1) * 512], po[64:65, :]
                        )

                # ---------- Z transpose + reciprocal ----------
                for ti in range(ti_lo, ti_hi):
                    pz = ps_sm.tile([128, 128], F32, tag="pt")
                    for hh in range(HL):
                        nc.tensor.matmul(
                            pz[0:128, hh:hh + 1],
                            lhsT=zrow_sb[hh][0:1, ti * 128:(ti + 1) * 128],
                            rhs=ident[0:1, 0:1],
                            start=True, stop=True,
                            skip_group_check=True,
                        )
                    nc.vector.reciprocal(
                        rz_sb[:, ti * HL:(ti + 1) * HL], pz[0:128, 0:HL]
                    )

                # ---------- Wo (+1/Z fold) -> attn partial -> AR ----------
                half = (ti_hi - ti_lo) // 2
                for c in range(2):
                    tis = range(ti_lo + c * half, ti_lo + (c + 1) * half)
                    for ti in tis:
                        rzA = rz_sb[:, ti * HL: ti * HL + 1]
                        rzB = rz_sb[:, ti * HL + 1: ti * HL + 2]
                        for dc in range(DC):
                            pA = ps_tmp.tile([128, 512], F32, tag="ptmp")
                            pB = ps_tmp.tile([128, 512], F32, tag="ptmp")
                            nc.tensor.matmul(
                                pA[:],
                                lhsT=oTu_sb[0:64, ti * 128:(ti + 1) * 128],
                                rhs=wo_sb[0:64, dc * 512:(dc + 1) * 512],
                                start=True, stop=True,
                            )
                            nc.tensor.matmul(
                                pB[:],
                                lhsT=oTu_sb[64:128, ti * 128:(ti + 1) * 128],
                                rhs=wo_sb[64:128, dc * 512:(dc + 1) * 512],
                                start=True, stop=True,
                            )
                            u = arp.tile([128, 512], F32, tag="u", bufs=2)
                            nc.vector.tensor_scalar_mul(u[:], pB[:], rzB)
                            a = arp.tile([128, 512], CC_DT, tag="a")
                            nc.vector.scalar_tensor_tensor(
                                a[:], in0=pA[:], scalar=rzA, in1=u[:],
                                op0=OP.mult, op1=OP.add,
                            )
                            nc.sync.dma_start(
                                out=ccin[("attn", lyr, c)][
                                    (ti - ti_lo - c * half) * 128:
                                    (ti - ti_lo - c * half + 1) * 128,
                                    dc * 512:(dc + 1) * 512,
                                ],
                                in_=a[:],
                            )
                    do_allreduce("attn", lyr, c)

                # ---------- rms2 (query range only) ----------
                rms_feature(nT_sb, qc_lo, qc_hi)

                # ---------- FFN up (wi0|wi1), gelu*gate -> gT (feature-major) ----------
                for f3 in range(F3):
                    for qc in range(qc_lo, qc_hi):
                        pg0 = ps_acc.tile([128, 512], F32, tag="pacc")
                        pg1 = ps_tmp.tile([128, 512], F32, tag="ptmp")
                        for kd in range(KD):
                            rhs = nT_sb[:, kd * 2048 + qc * 512: kd * 2048 + qc * 512 + 512]
                            nc.tensor.matmul(
                                pg0[:],
                                lhsT=wi_sb[:, kd * 2 * FL + f3 * 128: kd * 2 * FL + (f3 + 1) * 128],
                                rhs=rhs,
                                start=(kd == 0), stop=(kd == KD - 1),
                                skip_group_check=True,
                            )
                            nc.tensor.matmul(
                                pg1[:],
                                lhsT=wi_sb[:, kd * 2 * FL + FL + f3 * 128: kd * 2 * FL + FL + (f3 + 1) * 128],
                                rhs=rhs,
                                start=(kd == 0), stop=(kd == KD - 1),
                                skip_group_check=True,
                            )
                        gt = ep.tile([128, 512], F32, tag="gt", bufs=2)
                        nc.scalar.activation(gt[:], pg0[:], AF.Gelu_apprx_tanh)
                        nc.vector.tensor_mul(
                            gT_sb[:, f3 * 2048 + qc * 512: f3 * 2048 + (qc + 1) * 512],
                            gt[:], pg1[:],
                        )

                # ---------- FFN down -> partial -> AR ----------
                for c in range(2):
                    tis = range(ti_lo + c * half, ti_lo + (c + 1) * half)
                    for i, ti in enumerate(tis):
                        for dc in range(DC):
                            pf = ps_tmp.tile([128, 512], F32, tag="ptmp")
                            for f3 in range(F3):
                                nc.tensor.matmul(
                                    pf[:],
                                    lhsT=gT_sb[:, f3 * 2048 + ti * 128: f3 * 2048 + (ti + 1) * 128],
                                    rhs=wf_sb[:, f3 * 1024 + dc * 512: f3 * 1024 + (dc + 1) * 512],
                                    start=(f3 == 0), stop=(f3 == F3 - 1),
                                )
                            a = arp.tile([128, 512], CC_DT, tag="a")
                            nc.scalar.copy(a[:], pf[:])
                            nc.sync.dma_start(
                                out=ccin[("ffn", lyr, c)][
                                    i * 128:(i + 1) * 128,
                                    dc * 512:(dc + 1) * 512,
                                ],
                                in_=a[:],
                            )
                    do_allreduce("ffn", lyr, c)

            # ---------- final rms on second half, output (feature-major) ----------
            for qc in range(QC // 2, QC):
                ssq = ps_acc.tile([128, 512], F32, tag="pacc")
                for kd in range(KD):
                    sl = slice(kd * 2048 + qc * 512, kd * 2048 + qc * 512 + 512)
                    sqt = sqp.tile([128, 512], F16, tag="sq")
                    nc.vector.tensor_mul(sqt[:], hT_sb[:, sl], hT_sb[:, sl])
                    nc.tensor.matmul(
                        ssq[:], lhsT=ones_sb[:], rhs=sqt[:],
                        start=(kd == 0), stop=(kd == KD - 1),
                    )
                lno = stat.tile([128, 512], F32, tag="lno")
                nc.scalar.activation(
                    lno[:], ssq[:], AF.Ln, bias=eps_sb[:], scale=1.0 / D
                )
                rstd = stat.tile([128, 512], F16, tag="rstd")
                nc.scalar.activation(rstd[:], lno[:], AF.Exp, scale=-0.5)
                for kd in range(KD):
                    sl = slice(kd * 2048 + qc * 512, kd * 2048 + qc * 512 + 512)
                    o = arp.tile([128, 512], F32, tag="u", bufs=2)
                    nc.vector.tensor_mul(o[:], hT_sb[:, sl], rstd[:])
                    nc.sync.dma_start(
                        out=out[kd * 128:(kd + 1) * 128,
                                (qc - QC // 2) * 512:(qc - QC // 2 + 1) * 512],
                        in_=o[:],
                    )

    _fix_drain_waits(nc)
    return nc


# ---------------- host side ----------------

def _rel_bucket_np(rel):
    """numpy replica of reference _rel_bucket (int32/float32 semantics)."""
    nb = NB // 2
    ret = (rel > 0).astype(np.int32) * nb
    arel = np.abs(rel)
    max_exact = nb // 2
    t = np.log(np.maximum(arel, 1).astype(np.float32) / np.float32(max_exact))
    t = t / np.float32(np.log(MAXD / max_exact)) * np.float32(nb - max_exact)
    large = max_exact + t.astype(np.int32)
    large = np.minimum(large, nb - 1)
    return ret + np.where(arel < max_exact, arel.astype(np.int32), large)


def _build_mt(rel_bias, core):
    """exp(bias) master table [HL, 128, 3968] for this core's heads."""
    d = np.arange(-(T - 1), T, dtype=np.int64)          # k - q in [-2047, 2047]
    buckets = _rel_bucket_np(d)                          # [4095]
    p = np.arange(128)[:, None]
    i = np.arange(3968)[None, :]
    idx = 3967 + p - i                                   # in [0, 4094]
    mts = []
    for hh in range(HL):
        head = core * HL + hh
        toep = rel_bias[buckets, head].astype(np.float32)  # [4095]
        mts.append(np.exp(toep)[idx])
    return np.stack(mts).astype(ml_dtypes.bfloat16)


_prog_cache = {}


def kernel(**inputs):
    input_ids = np.asarray(inputs["input_ids"]).astype(np.int64)
    memory = np.asarray(inputs["memory"], dtype=np.float32)
    embed = np.asarray(inputs["embed"], dtype=np.float32)
    Wq = np.asarray(inputs["Wq"], dtype=np.float32)
    Wk = np.asarray(inputs["Wk"], dtype=np.float32)
    Wv = np.asarray(inputs["Wv"], dtype=np.float32)
    Wo = np.asarray(inputs["Wo"], dtype=np.float32)
    ln1 = np.asarray(inputs["ln1"], dtype=np.float32)
    ln2 = np.asarray(inputs["ln2"], dtype=np.float32)
    wi0 = np.asarray(inputs["wi0"], dtype=np.float32)
    wi1 = np.asarray(inputs["wi1"], dtype=np.float32)
    wo_ff = np.asarray(inputs["wo_ff"], dtype=np.float32)
    final_ln = np.asarray(inputs["final_ln"], dtype=np.float32)
    rel_bias = np.asarray(inputs["rel_bias"], dtype=np.float32)

    bf = np.float16

    x = embed[input_ids[0]]                      # [S, D]
    h0 = np.concatenate([memory[0], x], axis=0)  # [T, D] f32
    h0T = np.ascontiguousarray(h0.T).astype(bf)  # [D, T] f16

    in_maps = []
    for c in range(NCORES):
        hs = slice(c * DH, (c + 1) * DH)
        fs = slice(c * FLR, (c + 1) * FLR)
        wq_c = (ln1[:, :, None] * Wq)[:, :, hs].astype(bf)          # [L, D, DH]
        wk_c = (ln1[:, :, None] * Wk)[:, :, hs].astype(bf)
        wv_c = (ln1[:, :, None] * Wv)[:, :, hs].astype(bf)
        wo_c = Wo[:, hs, :].astype(ml_dtypes.bfloat16)               # [L, DH, D]
        wi_c = np.zeros((L, D, 2 * FL), np.float32)
        wi_c[:, :, :FLR] = (ln2[:, :, None] * wi0)[:, :, fs]
        wi_c[:, :, FL:FL + FLR] = (ln2[:, :, None] * wi1)[:, :, fs]
        wf_c = np.zeros((L, FL, D), np.float32)
        wf_c[:, :FLR, :] = wo_ff[:, fs, :]
        in_maps.append({
            "h0T": h0T,
            "WQ": np.ascontiguousarray(wq_c),
            "WK": np.ascontiguousarray(wk_c),
            "WV": np.ascontiguousarray(wv_c),
            "WO": np.ascontiguousarray(wo_c),
            "WI": wi_c.astype(bf),
            "WF": wf_c.astype(bf),
            "MT": _build_mt(rel_bias, c),
        })

    if "nc" not in _prog_cache:
        _prog_cache["nc"] = build_program()
    nc = _prog_cache["nc"]
    _prog_cache["in_maps"] = in_maps

    res = run_bass_kernel_spmd(nc, in_maps, list(range(NCORES)))
    hidT = res.results[0]["out"]                 # [D, S] normalized, unweighted
    outp = hidT.T * final_ln[None, :] + memory[0]
    return outp[None].astype(np.float32)


if __name__ == "__main__":
    rng = np.random.default_rng(0)
    fake = {
        "input_ids": rng.integers(0, V, (B, S)),
        "memory": rng.standard_normal((B, S, D), dtype=np.float32),
        "embed": rng.standard_normal((V, D), dtype=np.float32) * 0.02,
        "Wq": rng.standard_normal((L, D, H * DK), dtype=np.float32) * 0.02,
        "Wk": rng.standard_normal((L, D, H * DK), dtype=np.float32) * 0.02,
        "Wv": rng.standard_normal((L, D, H * DK), dtype=np.float32) * 0.02,
        "Wo": rng.standard_normal((L, H * DK, D), dtype=np.float32) * 0.02,
        "ln1": np.ones((L, D), np.float32),
        "ln2": np.ones((L, D), np.float32),
        "wi0": rng.standard_normal((L, D, DFF), dtype=np.float32) * 0.02,
        "wi1": rng.standard_normal((L, D, DFF), dtype=np.float32) * 0.02,
        "wo_ff": rng.standard_normal((L, DFF, D), dtype=np.float32) * 0.02,
        "final_ln": np.ones((D,), np.float32),
        "rel_bias": rng.standard_normal((NB, H), dtype=np.float32) * 0.02,
    }
    o = kernel(**fake)
    print("out", o.shape, o.dtype, np.abs(o).mean())
